# revision 1
# baseline (speedup 1.0000x reference)
"""CDMF segment-reduce kernel for 8 Trainium2 NeuronCores.

Strategy
--------
Host (cheap, index-only + one big gather):
  * stable-sort rows by user id; cut the 100k rows into 8 shards at user
    boundaries ("expert-style sharding of user segments") so each core owns a
    disjoint user range -> no cross-core reduction needed at all.
  * pad every shard to NT*128 rows (mask=0 rows contribute exactly 0).
  * pre-gather q = item_emb[items] per shard.
  * build per-tile one-hot matrices mapping the 128 rows of a tile to the
    user-slots of a PSUM "bank" (bank b = users first seen in tile b).

Device (one SPMD program on 8 cores):
  * stream R tiles [128 rows, 50, 64] (perfect per-partition-contiguous DMA),
    DVE multiply by w, DVE segmented reduce over d -> Z [128, 50]
  * threshold/mask -> per-row weight wt
  * PE one-hot matmuls accumulate per-user [sum wt*q | sum wt] (N=129) into
    PSUM banks; ACT flushes each bank to SBUF
  * transposed one-hot matmuls gather num[user]/den[user] back per row,
    reciprocal + fused (num*rec)*q multiply + reduce -> r.
"""

import numpy as np

import concourse.bass as bass
import concourse.tile as tile
from concourse import bacc, mybir
from concourse.bass_utils import run_bass_kernel_spmd

N_CORES = 8
TAU = 0.01
S = 50          # seq_len
D = 64          # n_features
E = 128         # emb_dim
F32 = mybir.dt.float32


# ----------------------------------------------------------------------------
# host-side preprocessing
# ----------------------------------------------------------------------------

def _preprocess(users, items, R_ui, mask, w, item_emb):
    n = users.shape[0]
    perm = np.argsort(users, kind="stable")
    users_s = users[perm]

    # shard cuts at user boundaries
    cuts = [0]
    for c in range(1, N_CORES):
        t = round(c * n / N_CORES)
        while 0 < t < n and users_s[t] == users_s[t - 1]:
            t += 1
        cuts.append(min(t, n))
    cuts.append(n)
    sizes = [cuts[c + 1] - cuts[c] for c in range(N_CORES)]
    NT = max(1, int(np.ceil(max(sizes) / 128)))
    NPAD = NT * 128

    q_full = item_emb[items]  # [n, E]

    in_maps = []
    metas = []
    wrep = np.ascontiguousarray(
        np.broadcast_to(w[None, None, :], (128, S, D)), dtype=np.float32
    )
    for c in range(N_CORES):
        lo, hi = cuts[c], cuts[c + 1]
        nc_rows = hi - lo
        p = perm[lo:hi]

        Rp = np.zeros((NPAD, S, D), np.float32)
        Rp[:nc_rows] = R_ui[p]

        mk = np.zeros((NPAD, S), np.float32)
        mk[:nc_rows] = mask[p]
        maskw = np.ascontiguousarray(mk.reshape(NT, 128, S).transpose(1, 0, 2))
        cntw = np.ascontiguousarray(maskw.sum(-1))  # [128, NT]
        # fast path (alpha=beta=gamma=1): wt = (sum_s mask*Wv) * cnt, so
        # pre-scaling the mask by cnt lets one fused op produce wt directly
        maskc = np.ascontiguousarray(maskw * cntw[:, :, None])

        qp = np.zeros((NPAD, E), np.float32)
        qp[:nc_rows] = q_full[p]
        qw = np.ascontiguousarray(qp.reshape(NT, 128, E).transpose(1, 0, 2))

        # users per padded row; pads take the last real user (wt=0 -> no-op)
        u = np.empty(NPAD, np.int64)
        u[:nc_rows] = users_s[lo:hi]
        u[nc_rows:] = u[nc_rows - 1] if nc_rows > 0 else 0

        # bank = tile where a user first appears; slot = rank within that bank
        first_tile = {}
        slot = {}
        bank_counts = [0] * NT
        for i in range(NPAD):
            uu = u[i]
            if uu not in first_tile:
                t = i // 128
                first_tile[uu] = t
                slot[uu] = bank_counts[t]
                bank_counts[t] += 1
        assert max(bank_counts) <= 128, f"bank overflow {max(bank_counts)}"

        oh_own = np.zeros((NT, 128, 128), np.float32)
        oh_nxt = np.zeros((NT, 128, 128), np.float32)
        for i in range(NPAD):
            t, k = divmod(i, 128)
            uu = u[i]
            ft = first_tile[uu]
            if ft == t:
                oh_own[t, k, slot[uu]] = 1.0
            else:
                # sorted rows: a user spans at most 2 consecutive tiles
                assert ft == t - 1, (ft, t)
                oh_nxt[ft, k, slot[uu]] = 1.0
        ohT_own = oh_own.transpose(0, 2, 1)
        ohT_nxt = oh_nxt.transpose(0, 2, 1)
        # packed pairs: [prev-tile closer | own] for segment mms,
        # [from-prev-bank | from-own-bank] for gather mms
        ohs_seg = np.zeros((NT, 128, 256), np.float32)
        ohs_seg[1:, :, 0:128] = oh_nxt[:-1]
        ohs_seg[:, :, 128:256] = oh_own
        ohs_gat = np.zeros((NT, 128, 256), np.float32)
        ohs_gat[1:, :, 0:128] = ohT_nxt[:-1]
        ohs_gat[:, :, 128:256] = ohT_own

        in_maps.append(
            {
                "Rp": Rp,
                "maskw": maskw,
                "maskc": maskc,
                "cntw": cntw,
                "qw": qw,
                "wrep": wrep,
                "ohs_seg": ohs_seg,
                "ohs_gat": ohs_gat,
            }
        )
        metas.append((p, nc_rows))
    return in_maps, metas, NT


# ----------------------------------------------------------------------------
# device program
# ----------------------------------------------------------------------------

def build_program(NT, alpha=1.0, beta=1.0, gamma=1.0):
    nc = bacc.Bacc(
        "TRN2", target_bir_lowering=False, debug=False, num_devices=N_CORES
    )
    NPAD = NT * 128

    Rp = nc.dram_tensor("Rp", [NPAD, S, D], F32, kind="ExternalInput")
    maskw = nc.dram_tensor("maskw", [128, NT, S], F32, kind="ExternalInput")
    maskc = nc.dram_tensor("maskc", [128, NT, S], F32, kind="ExternalInput")
    cntw = nc.dram_tensor("cntw", [128, NT], F32, kind="ExternalInput")
    qw = nc.dram_tensor("qw", [128, NT, E], F32, kind="ExternalInput")
    wrep = nc.dram_tensor("wrep", [128, S, D], F32, kind="ExternalInput")
    ohs_seg = nc.dram_tensor("ohs_seg", [NT, 128, 256], F32, kind="ExternalInput")
    ohs_gat = nc.dram_tensor("ohs_gat", [NT, 128, 256], F32, kind="ExternalInput")
    r_out = nc.dram_tensor("r_out", [128, NT], F32, kind="ExternalOutput")

    fast = (alpha == 1.0) and (beta == 1.0) and (gamma == 1.0)
    AF = mybir.ActivationFunctionType

    with tile.TileContext(nc) as tc:
        with (
            tc.tile_pool(name="const", bufs=1) as constp,
            tc.tile_pool(name="rpool", bufs=4) as rpool,
            tc.tile_pool(name="ypool", bufs=1) as ypool,
            tc.tile_pool(name="zpool", bufs=6) as zpool,
            tc.tile_pool(name="small", bufs=8) as small,
            tc.tile_pool(name="qpool", bufs=6) as qpool,
            tc.tile_pool(name="xpool", bufs=6) as xpool,
            tc.tile_pool(name="ohpool", bufs=6) as ohpool,
            tc.tile_pool(name="ohgpool", bufs=6) as ohgpool,
            tc.tile_pool(name="banks", bufs=1) as bankp,
            tc.tile_pool(name="psum_seg", bufs=3, space="PSUM") as pseg,
            tc.tile_pool(name="psum_gat", bufs=3, space="PSUM") as pgat,
        ):
            w_sb = constp.tile([128, S, D], F32)
            nc.sync.dma_start(w_sb[:], wrep[:, :, :])
            mask_sb = constp.tile([128, NT, S], F32)
            nc.sync.dma_start(mask_sb[:], maskc[:, :, :] if fast else maskw[:, :, :])
            if not fast:
                cnt_sb = constp.tile([128, NT], F32)
                nc.sync.dma_start(cnt_sb[:], cntw[:, :])
            den_sb = constp.tile([128, NT], F32)
            wt_sb = constp.tile([128, NT], F32)
            r_sb = constp.tile([128, NT], F32)
            bank_sb = bankp.tile([128, NT, 129], F32)

            x_tiles = [None] * NT
            q_groups = {}
            bank_ps = [None] * NT

            # ---- phase A+B interleaved: Z -> wt -> X -> segment matmuls ----
            for t in range(NT):
                rt = rpool.tile([128, S, D], F32)
                nc.sync.dma_start(rt[:], Rp[t * 128 : (t + 1) * 128, :, :])
                y = ypool.tile([128, S, D], F32)
                nc.vector.tensor_mul(y[:], rt[:], w_sb[:])
                z = zpool.tile([128, S], F32)
                nc.vector.tensor_reduce(
                    z[:], y[:], axis=mybir.AxisListType.X, op=mybir.AluOpType.add
                )
                wp = zpool.tile([128, S], F32)
                wt_col = wt_sb[:, t : t + 1]
                if fast:
                    # wt = sum_s (max(z, tau) * mask*cnt), fused in one DVE op
                    nc.vector.scalar_tensor_tensor(
                        wp[:], z[:], TAU, mask_sb[:, t, :],
                        op0=mybir.AluOpType.max, op1=mybir.AluOpType.mult,
                        accum_out=wt_col,
                    )
                else:
                    nc.vector.tensor_scalar_max(z[:], z[:], TAU)
                    # z <- exp(alpha * ln z)   (z >= TAU > 0)
                    nc.scalar.activation(z[:], z[:], AF.Log)
                    nc.scalar.activation(z[:], z[:], AF.Exp, scale=float(alpha))
                    nc.vector.tensor_mul(wp[:], z[:], mask_sb[:, t, :])
                if not fast:
                    a_col = small.tile([128, 1], F32)
                    nc.vector.tensor_reduce(
                        a_col[:], wp[:], axis=mybir.AxisListType.X,
                        op=mybir.AluOpType.add,
                    )
                    # wt = (A^(1/alpha) * cnt^beta)^gamma
                    #    = exp(gamma*(ln(A)/alpha + beta*ln(cnt)))
                    la = small.tile([128, 1], F32)
                    nc.scalar.activation(la[:], a_col[:], AF.Log)
                    lc = small.tile([128, 1], F32)
                    nc.scalar.activation(lc[:], cnt_sb[:, t : t + 1], AF.Log)
                    # la <- (lc * alpha*beta) + la ; wt = exp((gamma/alpha)*la)
                    nc.vector.scalar_tensor_tensor(
                        la[:], lc[:], float(alpha * beta), la[:],
                        op0=mybir.AluOpType.mult, op1=mybir.AluOpType.add,
                    )
                    nc.scalar.activation(
                        wt_col, la[:], AF.Exp, scale=float(gamma / alpha)
                    )

                # X_t = [wt*q | wt]
                g, j = divmod(t, 3)
                if j == 0:
                    ng = min(3, NT - t)
                    qg = qpool.tile([128, 3, E], F32)
                    nc.sync.dma_start(
                        qg[:, 0:ng, :], qw[:, t : t + ng, :]
                    )
                    q_groups[g] = qg
                qt = q_groups[g][:, j, :]
                xt = xpool.tile([128, 129], F32)
                nc.scalar.mul(xt[:, 0:E], qt, wt_col)
                nc.scalar.copy(xt[:, E : E + 1], wt_col)
                x_tiles[t] = xt

                oh2 = ohpool.tile([128, 256], F32)
                nc.sync.dma_start(oh2[:], ohs_seg[t, :, :])
                # leftovers of this tile into previous tile's bank (closes it)
                if t >= 1:
                    nc.tensor.matmul(
                        bank_ps[t - 1][:], oh2[:, 0:128], xt[:],
                        start=False, stop=True,
                    )
                    nc.scalar.copy(bank_sb[:, t - 1, :], bank_ps[t - 1][:])
                ohA = oh2[:, 128:256]
                ps = pseg.tile([128, 129], F32)
                bank_ps[t] = ps
                last = t == NT - 1
                nc.tensor.matmul(ps[:], ohA[:], xt[:], start=True, stop=last)
                if last:
                    nc.scalar.copy(bank_sb[:, t, :], ps[:])

            # ---- phase C: gather num/den per row, divide, dot with q ----
            NG = (NT + 2) // 3
            for g in range(NG):
                t0 = g * 3
                ng = min(3, NT - t0)
                gp = pgat.tile([128, 3, 129], F32)
                for j in range(ng):
                    t = t0 + j
                    g2 = ohgpool.tile([128, 256], F32)
                    nc.scalar.dma_start(g2[:], ohs_gat[t, :, :])
                    if t >= 1:
                        nc.tensor.matmul(
                            gp[:, j, :], g2[:, 0:128], bank_sb[:, t - 1, :],
                            start=True, stop=False,
                        )
                        nc.tensor.matmul(
                            gp[:, j, :], g2[:, 128:256], bank_sb[:, t, :],
                            start=False, stop=True,
                        )
                    else:
                        nc.tensor.matmul(
                            gp[:, j, :], g2[:, 128:256], bank_sb[:, t, :],
                            start=True, stop=True,
                        )
                nc.scalar.copy(
                    den_sb[:, t0 : t0 + ng],
                    gp[:, 0:ng, E : E + 1].rearrange("p a b -> p (a b)"),
                )
                pq = zpool.tile([128, 3, E], F32)
                nc.vector.tensor_mul(
                    pq[:, 0:ng, :], gp[:, 0:ng, 0:E], q_groups[g][:, 0:ng, :]
                )
                nc.vector.tensor_reduce(
                    r_sb[:, t0 : t0 + ng], pq[:, 0:ng, :],
                    axis=mybir.AxisListType.X, op=mybir.AluOpType.add,
                )

            # r = (sum_e num*q) / den, one divide for all tiles
            rec_all = small.tile([128, NT], F32)
            nc.vector.reciprocal(rec_all[:], den_sb[:])
            nc.vector.tensor_mul(r_sb[:], r_sb[:], rec_all[:])

            nc.sync.dma_start(r_out[:, :], r_sb[:])

    nc.compile()
    return nc


# ----------------------------------------------------------------------------
# entry point
# ----------------------------------------------------------------------------

def kernel(users, items, R_ui, mask, w, item_emb, alpha, beta, gamma,
           _return_extras=False, _trace=False):
    users = np.asarray(users, np.int64)
    items = np.asarray(items, np.int64)
    R_ui = np.asarray(R_ui, np.float32)
    mask_b = np.asarray(mask)
    mask_f = mask_b.astype(np.float32)
    w = np.asarray(w, np.float32)
    item_emb = np.asarray(item_emb, np.float32)
    al = float(np.asarray(alpha).reshape(-1)[0])
    be = float(np.asarray(beta).reshape(-1)[0])
    ga = float(np.asarray(gamma).reshape(-1)[0])

    import time as _time

    t0 = _time.perf_counter()
    in_maps, metas, NT = _preprocess(users, items, R_ui, mask_f, w, item_emb)
    t1 = _time.perf_counter()
    nc = build_program(NT, al, be, ga)
    t2 = _time.perf_counter()
    res = run_bass_kernel_spmd(
        nc, in_maps, core_ids=list(range(N_CORES)), trace=_trace
    )
    t3 = _time.perf_counter()
    print(
        f"[kernel] preprocess {t1-t0:.1f}s  build+schedule {t2-t1:.1f}s  "
        f"compile+run {t3-t2:.1f}s"
    )

    n = users.shape[0]
    r = np.empty(n, np.float32)
    for c in range(N_CORES):
        p, nc_rows = metas[c]
        shard = res.results[c]["r_out"].T.reshape(-1)[:nc_rows]
        r[p] = shard
    if _return_extras:
        return r, res
    return r



# revision 3
# speedup vs baseline: 2.5968x; 2.5968x over previous
"""CDMF segment-reduce kernel for 8 Trainium2 NeuronCores.

Strategy
--------
Host (cheap, index-only + one big gather):
  * stable-sort rows by user id; cut the 100k rows into 8 shards at user
    boundaries ("expert-style sharding of user segments") so each core owns a
    disjoint user range -> no cross-core reduction needed at all.
  * pad every shard to NT*128 rows (mask=0 rows contribute exactly 0).
  * pre-gather q = item_emb[items] per shard, cast data to bf16.
  * transpose R into PE-friendly chunks RT[(s%2)*64+d, t, j, row] so the
    feature contraction runs on the tensor engine.
  * build per-tile one-hot matrices (fp8, values 0/1 exact) mapping the 128
    rows of a tile to user-slots of a PSUM "bank" (bank b = users first seen
    in tile b).

Device (one SPMD program on 8 cores):
  * PE: per tile, 25 accumulating matmuls lhsT=RT-chunk [K=128=(2 s x 64 d),
    M=128 rows], rhs=w-pattern [128, 50] -> Z [128 rows, 50 s] in PSUM.
    The w-pattern w[d] * delta(s == 2j+s') realizes Z = sum_d R*w per s.
  * DVE: one fused scalar_tensor_tensor (max(Z,tau) * maskc, accum) ->
    per-row weight wt; two tensor_scalar ops build X = [wt*q | wt].
  * PE one-hot matmuls accumulate per-user [sum wt*q | sum wt] (N=129) into
    PSUM banks; ACT flushes each bank to SBUF (bf16).
  * transposed one-hot matmuls gather num[user]/den[user] back per row;
    fused tensor_tensor_reduce computes sum_e num*q; reciprocal+mul -> r.
"""

import numpy as np
import ml_dtypes

import concourse.bass as bass
import concourse.tile as tile
from concourse import bacc, mybir
from concourse.bass_utils import run_bass_kernel_spmd

N_CORES = 8
TAU = 0.01
S = 50          # seq_len
D = 64          # n_features
E = 128         # emb_dim
NJ = S // 2     # PE k-chunks per tile (2 s-slices of 64 features each)
GR = 4          # R tiles per DMA
GO = 8          # one-hot tiles per DMA
F32 = mybir.dt.float32
BF16 = mybir.dt.bfloat16
FP8 = mybir.dt.float8e4

NP_BF16 = ml_dtypes.bfloat16
NP_FP8 = mybir.dt.np(FP8)


# ----------------------------------------------------------------------------
# host-side preprocessing
# ----------------------------------------------------------------------------

def _preprocess(users, items, R_ui, mask, w, item_emb):
    n = users.shape[0]
    perm = np.argsort(users, kind="stable")
    users_s = users[perm]

    # shard cuts at user boundaries
    cuts = [0]
    for c in range(1, N_CORES):
        t = round(c * n / N_CORES)
        while 0 < t < n and users_s[t] == users_s[t - 1]:
            t += 1
        cuts.append(min(t, n))
    cuts.append(n)
    sizes = [cuts[c + 1] - cuts[c] for c in range(N_CORES)]
    NT = max(1, int(np.ceil(max(sizes) / 128)))
    NPAD = NT * 128

    q_full = item_emb[items]  # [n, E]
    w_bf = np.asarray(w, NP_BF16)

    # w-pattern for the PE feature contraction: [128=(s'*64+d), NJ, 50]
    wpat = np.zeros((128, NJ, S), NP_BF16)
    for sp in range(2):
        for j in range(NJ):
            wpat[sp * 64:(sp + 1) * 64, j, 2 * j + sp] = w_bf

    in_maps = []
    metas = []
    for c in range(N_CORES):
        lo, hi = cuts[c], cuts[c + 1]
        nc_rows = hi - lo
        p = perm[lo:hi]

        # R^T chunks: RT[(s%2)*64+d, t, j, row] = R[row, 2j+(s%2), d]
        Rp = np.zeros((NPAD, S, D), NP_BF16)
        Rp[:nc_rows] = R_ui[p]
        RT = np.ascontiguousarray(
            Rp.reshape(NT, 128, NJ, 2, D).transpose(3, 4, 0, 2, 1)
        ).reshape(128, NT, NJ, 128)

        mk = np.zeros((NPAD, S), np.float32)
        mk[:nc_rows] = mask[p]
        maskw3 = mk.reshape(NT, 128, S).transpose(1, 0, 2)  # [128, NT, S]
        cntw = np.ascontiguousarray(maskw3.sum(-1), np.float32)  # [128, NT]
        # fast path (alpha=beta=gamma=1): wt = (sum_s mask*Wv) * cnt, so
        # pre-scaling the mask by cnt lets one fused op produce wt directly
        maskc = np.ascontiguousarray(
            maskw3 * cntw[:, :, None], dtype=NP_BF16
        )
        maskw = np.ascontiguousarray(maskw3, dtype=NP_BF16)

        qp = np.zeros((NPAD, E), NP_BF16)
        qp[:nc_rows] = q_full[p]
        qw = np.ascontiguousarray(qp.reshape(NT, 128, E).transpose(1, 0, 2))

        # users per padded row; pads take the last real user (wt=0 -> no-op)
        u = np.empty(NPAD, np.int64)
        u[:nc_rows] = users_s[lo:hi]
        u[nc_rows:] = u[nc_rows - 1] if nc_rows > 0 else 0

        # bank = tile where a user first appears; slot = rank within bank
        uniq, first_idx, inv = np.unique(u, return_index=True,
                                         return_inverse=True)
        ft = first_idx // 128                       # first tile per user
        gstart = np.searchsorted(ft, ft, side="left")
        slot = np.arange(len(uniq)) - gstart        # rank within its bank
        assert np.bincount(ft, minlength=NT).max() <= 128, "bank overflow"

        ii = np.arange(NPAD)
        tt, kk = ii // 128, ii % 128
        fr, sr = ft[inv], slot[inv]
        own = fr == tt
        prev = fr == tt - 1
        assert np.all(own | prev), "user spans >2 tiles (unexpected padding)"

        # packed pairs: [prev-tile closer | own] for segment matmuls,
        # [from-prev-bank | from-own-bank] (transposed) for gather matmuls
        seg = np.zeros((128, NT, 256), NP_FP8)
        gat = np.zeros((128, NT, 256), NP_FP8)
        seg[kk[own], tt[own], 128 + sr[own]] = 1.0
        seg[kk[prev], tt[prev], sr[prev]] = 1.0
        gat[sr[own], tt[own], 128 + kk[own]] = 1.0
        gat[sr[prev], tt[prev], kk[prev]] = 1.0

        in_maps.append(
            {
                "RT": RT,
                "maskw": maskw,
                "maskc": maskc,
                "cntw": cntw,
                "qw": qw,
                "wpat": wpat,
                "ohs_seg": seg,
                "ohs_gat": gat,
            }
        )
        metas.append((p, nc_rows))
    return in_maps, metas, NT


# ----------------------------------------------------------------------------
# device program
# ----------------------------------------------------------------------------

def build_program(NT, alpha=1.0, beta=1.0, gamma=1.0):
    nc = bacc.Bacc(
        "TRN2", target_bir_lowering=False, debug=False, num_devices=N_CORES
    )

    RT = nc.dram_tensor("RT", [128, NT, NJ, 128], BF16, kind="ExternalInput")
    maskw = nc.dram_tensor("maskw", [128, NT, S], BF16, kind="ExternalInput")
    maskc = nc.dram_tensor("maskc", [128, NT, S], BF16, kind="ExternalInput")
    cntw = nc.dram_tensor("cntw", [128, NT], F32, kind="ExternalInput")
    qw = nc.dram_tensor("qw", [128, NT, E], BF16, kind="ExternalInput")
    wpat = nc.dram_tensor("wpat", [128, NJ, S], BF16, kind="ExternalInput")
    ohs_seg = nc.dram_tensor("ohs_seg", [128, NT, 256], FP8, kind="ExternalInput")
    ohs_gat = nc.dram_tensor("ohs_gat", [128, NT, 256], FP8, kind="ExternalInput")
    r_out = nc.dram_tensor("r_out", [128, NT], F32, kind="ExternalOutput")

    fast = (alpha == 1.0) and (beta == 1.0) and (gamma == 1.0)
    AF = mybir.ActivationFunctionType

    NGR = (NT + GR - 1) // GR
    NGO = (NT + GO - 1) // GO

    with tile.TileContext(nc) as tc:
        with (
            tc.tile_pool(name="const", bufs=1) as constp,
            tc.tile_pool(name="rpool", bufs=2) as rpool,
            tc.tile_pool(name="zpool", bufs=2) as zpool,
            tc.tile_pool(name="small", bufs=8) as small,
            tc.tile_pool(name="xpool", bufs=4) as xpool,
            tc.tile_pool(name="ohpool", bufs=2) as ohpool,
            tc.tile_pool(name="ohgpool", bufs=2) as ohgpool,
            tc.tile_pool(name="banks", bufs=1) as bankp,
            tc.tile_pool(name="psum_z", bufs=2, space="PSUM") as pz,
            tc.tile_pool(name="psum_seg", bufs=3, space="PSUM") as pseg,
            tc.tile_pool(name="psum_gat", bufs=3, space="PSUM") as pgat,
        ):
            wpat_sb = constp.tile([128, NJ, S], BF16)
            nc.sync.dma_start(wpat_sb[:], wpat[:, :, :])
            mask_sb = constp.tile([128, NT, S], BF16)
            nc.sync.dma_start(mask_sb[:], maskc[:, :, :] if fast else maskw[:, :, :])
            qw_sb = constp.tile([128, NT, E], BF16)
            nc.sync.dma_start(qw_sb[:], qw[:, :, :])
            if not fast:
                cnt_sb = constp.tile([128, NT], F32)
                nc.sync.dma_start(cnt_sb[:], cntw[:, :])
            den_sb = constp.tile([128, NT], F32)
            wt_sb = constp.tile([128, NT], F32)
            rn_sb = constp.tile([128, NT], F32)
            r_sb = constp.tile([128, NT], F32)
            bank_sb = bankp.tile([128, NT, 129], BF16)

            r_groups = {}
            oh_groups = {}
            bank_ps = [None] * NT

            # ---- phase A+B: Z (PE) -> wt -> X -> segment matmuls ----
            for t in range(NT):
                g, gi = divmod(t, GR)
                if gi == 0:
                    gn = min(GR, NT - t)
                    rg = rpool.tile([128, GR, NJ, 128], BF16)
                    nc.sync.dma_start(rg[:, 0:gn, :, :], RT[:, t:t + gn, :, :])
                    r_groups[g] = rg
                og, ogi = divmod(t, GO)
                if ogi == 0:
                    ogn = min(GO, NT - t)
                    osg = ohpool.tile([128, GO, 256], FP8)
                    nc.sync.dma_start(osg[:, 0:ogn, :], ohs_seg[:, t:t + ogn, :])
                    oh_groups[og] = osg

                zps = pz.tile([128, S], F32)
                rg = r_groups[g]
                for j in range(NJ):
                    nc.tensor.matmul(
                        zps[:], rg[:, gi, j, :], wpat_sb[:, j, :],
                        start=(j == 0), stop=(j == NJ - 1),
                    )

                wt_col = wt_sb[:, t:t + 1]
                if fast:
                    # wt = sum_s (max(z, tau) * mask*cnt), one fused DVE op
                    wp = zpool.tile([128, S], BF16)
                    nc.vector.scalar_tensor_tensor(
                        wp[:], zps[:], TAU, mask_sb[:, t, :],
                        op0=mybir.AluOpType.max, op1=mybir.AluOpType.mult,
                        accum_out=wt_col,
                    )
                else:
                    z = zpool.tile([128, S], F32, tag="zf32")
                    nc.vector.tensor_scalar_max(z[:], zps[:], TAU)
                    # z <- exp(alpha * ln z)   (z >= TAU > 0)
                    nc.scalar.activation(z[:], z[:], AF.Log)
                    nc.scalar.activation(z[:], z[:], AF.Exp, scale=float(alpha))
                    wp = zpool.tile([128, S], F32, tag="wpf32")
                    nc.vector.tensor_mul(wp[:], z[:], mask_sb[:, t, :])
                    a_col = small.tile([128, 1], F32)
                    nc.vector.tensor_reduce(
                        a_col[:], wp[:], axis=mybir.AxisListType.X,
                        op=mybir.AluOpType.add,
                    )
                    # wt = (A^(1/alpha) * cnt^beta)^gamma
                    #    = exp(gamma*(ln(A)/alpha + beta*ln(cnt)))
                    la = small.tile([128, 1], F32)
                    nc.scalar.activation(la[:], a_col[:], AF.Log)
                    lc = small.tile([128, 1], F32)
                    nc.scalar.activation(lc[:], cnt_sb[:, t:t + 1], AF.Log)
                    nc.vector.scalar_tensor_tensor(
                        la[:], lc[:], float(alpha * beta), la[:],
                        op0=mybir.AluOpType.mult, op1=mybir.AluOpType.add,
                    )
                    nc.scalar.activation(
                        wt_col, la[:], AF.Exp, scale=float(gamma / alpha)
                    )

                # X_t = [wt*q | wt]
                xt = xpool.tile([128, 129], BF16)
                nc.vector.tensor_scalar_mul(xt[:, 0:E], qw_sb[:, t, :], wt_col)
                nc.vector.tensor_scalar_add(xt[:, E:E + 1], wt_col, 0.0)

                oh2 = oh_groups[og]
                # leftovers of this tile into previous tile's bank (closes it)
                if t >= 1:
                    nc.tensor.matmul(
                        bank_ps[t - 1][:], oh2[:, ogi, 0:128], xt[:],
                        start=False, stop=True,
                    )
                    nc.scalar.copy(bank_sb[:, t - 1, :], bank_ps[t - 1][:])
                ps = pseg.tile([128, 129], F32)
                bank_ps[t] = ps
                last = t == NT - 1
                nc.tensor.matmul(
                    ps[:], oh2[:, ogi, 128:256], xt[:], start=True, stop=last
                )
                if last:
                    nc.scalar.copy(bank_sb[:, t, :], ps[:])

            # ---- phase C: gather num/den per row, dot with q ----
            for t in range(NT):
                og, ogi = divmod(t, GO)
                if ogi == 0:
                    ogn = min(GO, NT - t)
                    ogt = ohgpool.tile([128, GO, 256], FP8)
                    nc.sync.dma_start(ogt[:, 0:ogn, :], ohs_gat[:, t:t + ogn, :])
                    oh_groups[("g", og)] = ogt
                g2 = oh_groups[("g", og)]
                gp = pgat.tile([128, 129], F32)
                if t >= 1:
                    nc.tensor.matmul(
                        gp[:], g2[:, ogi, 0:128], bank_sb[:, t - 1, :],
                        start=True, stop=False,
                    )
                    nc.tensor.matmul(
                        gp[:], g2[:, ogi, 128:256], bank_sb[:, t, :],
                        start=False, stop=True,
                    )
                else:
                    nc.tensor.matmul(
                        gp[:], g2[:, ogi, 128:256], bank_sb[:, t, :],
                        start=True, stop=True,
                    )
                nc.vector.tensor_scalar_add(
                    den_sb[:, t:t + 1], gp[:, E:E + 1], 0.0
                )
                # rnum = sum_e num[user] * q, fused multiply+accumulate
                pq = zpool.tile([128, E], BF16, tag="pq")
                nc.vector.scalar_tensor_tensor(
                    pq[:], gp[:, 0:E], 0.0, qw_sb[:, t, :],
                    op0=mybir.AluOpType.add, op1=mybir.AluOpType.mult,
                    accum_out=rn_sb[:, t:t + 1],
                )

            # r = rnum / den, one divide for all tiles
            rec_all = small.tile([128, NT], F32, tag="rec")
            nc.vector.reciprocal(rec_all[:], den_sb[:])
            nc.vector.tensor_mul(r_sb[:], rn_sb[:], rec_all[:])

            nc.sync.dma_start(r_out[:, :], r_sb[:])

    nc.compile()
    return nc


# ----------------------------------------------------------------------------
# entry point
# ----------------------------------------------------------------------------

def kernel(users, items, R_ui, mask, w, item_emb, alpha, beta, gamma,
           _return_extras=False, _trace=False):
    users = np.asarray(users, np.int64)
    items = np.asarray(items, np.int64)
    R_ui = np.asarray(R_ui, np.float32)
    mask_b = np.asarray(mask)
    mask_f = mask_b.astype(np.float32)
    w = np.asarray(w, np.float32)
    item_emb = np.asarray(item_emb, np.float32)
    al = float(np.asarray(alpha).reshape(-1)[0])
    be = float(np.asarray(beta).reshape(-1)[0])
    ga = float(np.asarray(gamma).reshape(-1)[0])

    import time as _time

    t0 = _time.perf_counter()
    in_maps, metas, NT = _preprocess(users, items, R_ui, mask_f, w, item_emb)
    t1 = _time.perf_counter()
    nc = build_program(NT, al, be, ga)
    t2 = _time.perf_counter()
    res = run_bass_kernel_spmd(
        nc, in_maps, core_ids=list(range(N_CORES)), trace=_trace
    )
    t3 = _time.perf_counter()
    print(
        f"[kernel] preprocess {t1-t0:.1f}s  build+schedule {t2-t1:.1f}s  "
        f"compile+run {t3-t2:.1f}s"
    )

    n = users.shape[0]
    r = np.empty(n, np.float32)
    for c in range(N_CORES):
        p, nc_rows = metas[c]
        shard = res.results[c]["r_out"].T.reshape(-1)[:nc_rows]
        r[p] = shard
    if _return_extras:
        return r, res
    return r


# revision 8
# speedup vs baseline: 4.2215x; 1.6257x over previous
"""CDMF segment-reduce kernel for 8 Trainium2 NeuronCores.

Strategy
--------
Host (cheap, index-only + one big gather):
  * stable-sort rows by user id; cut the 100k rows into 8 shards at user
    boundaries ("expert-style sharding of user segments") so each core owns a
    disjoint user range -> no cross-core reduction needed at all.
  * pad every shard to NT*128 rows (mask=0 rows contribute exactly 0).
  * pre-gather q = item_emb[items] per shard, cast data to bf16.
  * transpose R into PE-friendly chunks RT[(s%2)*64+d, t, j, row] so the
    feature contraction runs on the tensor engine.
  * build per-tile one-hot matrices (fp8, values 0/1 exact) mapping the 128
    rows of a tile to user-slots of a PSUM "bank" (bank b = users first seen
    in tile b).

Device (one SPMD program on 8 cores):
  * PE: per tile, 25 accumulating matmuls lhsT=RT-chunk [K=128=(2 s x 64 d),
    M=128 rows], rhs=w-pattern [128, 50] -> Z [128 rows, 50 s] in PSUM.
    The w-pattern w[d] * delta(s == 2j+s') realizes Z = sum_d R*w per s.
  * DVE: one fused scalar_tensor_tensor (max(Z,tau) * maskc, accum) ->
    per-row weight wt; two tensor_scalar ops build X = [wt*q | wt].
  * PE one-hot matmuls accumulate per-user [sum wt*q | sum wt] (N=129) into
    PSUM banks; ACT flushes each bank to SBUF (bf16).
  * transposed one-hot matmuls gather num[user]/den[user] back per row;
    fused tensor_tensor_reduce computes sum_e num*q; reciprocal+mul -> r.
"""

import numpy as np
import ml_dtypes

import concourse.bass as bass
import concourse.tile as tile
from concourse import bacc, mybir
from concourse.bass_utils import run_bass_kernel_spmd

N_CORES = 8
TAU = 0.01
S = 50          # seq_len
D = 64          # n_features
E = 128         # emb_dim
NJ = S // 2     # PE k-chunks per tile (2 s-slices of 64 features each)
GR = 4          # R tiles per DMA
GO = 8          # one-hot tiles per DMA
F32 = mybir.dt.float32
BF16 = mybir.dt.bfloat16
FP8 = mybir.dt.float8e4
R_DT = FP8      # R stream dtype: fp8e4m3 keeps max-normalized err ~8e-3

NP_BF16 = ml_dtypes.bfloat16
NP_FP8 = mybir.dt.np(FP8)
NP_R = mybir.dt.np(R_DT)


# ----------------------------------------------------------------------------
# host-side preprocessing
# ----------------------------------------------------------------------------

def _preprocess(users, items, R_ui, mask, w, item_emb):
    n = users.shape[0]
    perm = np.argsort(users, kind="stable")
    users_s = users[perm]

    # shard cuts at user boundaries
    cuts = [0]
    for c in range(1, N_CORES):
        t = round(c * n / N_CORES)
        while 0 < t < n and users_s[t] == users_s[t - 1]:
            t += 1
        cuts.append(min(t, n))
    cuts.append(n)
    sizes = [cuts[c + 1] - cuts[c] for c in range(N_CORES)]
    NT = max(1, int(np.ceil(max(sizes) / 128)))
    NPAD = NT * 128

    q_full = item_emb[items]  # [n, E]
    w_bf = np.asarray(w, NP_BF16)

    # w-pattern for the PE feature contraction: [128=(s'*64+d), NJ, 50]
    wpat = np.zeros((128, NJ, S), NP_BF16)
    for sp in range(2):
        for j in range(NJ):
            wpat[sp * 64:(sp + 1) * 64, j, 2 * j + sp] = w_bf

    in_maps = []
    metas = []
    for c in range(N_CORES):
        lo, hi = cuts[c], cuts[c + 1]
        nc_rows = hi - lo
        p = perm[lo:hi]

        # R^T chunks: RT[(s%2)*64+d, t, j, row] = R[row, 2j+(s%2), d]
        Rp = np.zeros((NPAD, S, D), NP_R)
        Rp[:nc_rows] = R_ui[p]
        RT = np.ascontiguousarray(
            Rp.reshape(NT, 128, NJ, 2, D).transpose(3, 4, 0, 2, 1)
        ).reshape(128, NT, NJ, 128)

        mk = np.zeros((NPAD, S), np.float32)
        mk[:nc_rows] = mask[p]
        maskw3 = mk.reshape(NT, 128, S).transpose(1, 0, 2)  # [128, NT, S]
        cntw = np.ascontiguousarray(maskw3.sum(-1), np.float32)  # [128, NT]
        # fast path (alpha=beta=gamma=1): wt = (sum_s mask*Wv) * cnt, so
        # pre-scaling the mask by cnt lets one fused op produce wt directly
        maskc = np.ascontiguousarray(
            maskw3 * cntw[:, :, None], dtype=NP_BF16
        )
        maskw = np.ascontiguousarray(maskw3, dtype=NP_BF16)

        qp = np.zeros((NPAD, E), NP_BF16)
        qp[:nc_rows] = q_full[p]
        qw = np.ascontiguousarray(qp.reshape(NT, 128, E).transpose(1, 0, 2))

        # users per padded row; pads take the last real user (wt=0 -> no-op)
        u = np.empty(NPAD, np.int64)
        u[:nc_rows] = users_s[lo:hi]
        u[nc_rows:] = u[nc_rows - 1] if nc_rows > 0 else 0

        # bank = tile where a user first appears; slot = rank within bank
        uniq, first_idx, inv = np.unique(u, return_index=True,
                                         return_inverse=True)
        ft = first_idx // 128                       # first tile per user
        gstart = np.searchsorted(ft, ft, side="left")
        slot = np.arange(len(uniq)) - gstart        # rank within its bank
        assert np.bincount(ft, minlength=NT).max() <= 128, "bank overflow"

        ii = np.arange(NPAD)
        tt, kk = ii // 128, ii % 128
        fr, sr = ft[inv], slot[inv]
        own = fr == tt
        prev = fr == tt - 1
        assert np.all(own | prev), "user spans >2 tiles (unexpected padding)"

        # packed pairs: [prev-tile closer | own] for segment matmuls,
        # [from-prev-bank | from-own-bank] (transposed) for gather matmuls
        seg = np.zeros((128, NT, 256), NP_FP8)
        gat = np.zeros((128, NT, 256), NP_FP8)
        seg[kk[own], tt[own], 128 + sr[own]] = 1.0
        seg[kk[prev], tt[prev], sr[prev]] = 1.0
        gat[sr[own], tt[own], 128 + kk[own]] = 1.0
        gat[sr[prev], tt[prev], kk[prev]] = 1.0

        in_maps.append(
            {
                "RT": RT,
                "maskw": maskw,
                "maskc": maskc,
                "cntw": cntw,
                "qw": qw,
                "wpat": wpat,
                "ohs_seg": seg,
                "ohs_gat": gat,
            }
        )
        metas.append((p, nc_rows))
    return in_maps, metas, NT


# ----------------------------------------------------------------------------
# device program
# ----------------------------------------------------------------------------

def build_program(NT, alpha=1.0, beta=1.0, gamma=1.0):
    nc = bacc.Bacc(
        "TRN2", target_bir_lowering=False, debug=False, num_devices=N_CORES
    )

    RT = nc.dram_tensor("RT", [128, NT, NJ, 128], R_DT, kind="ExternalInput")
    maskw = nc.dram_tensor("maskw", [128, NT, S], BF16, kind="ExternalInput")
    maskc = nc.dram_tensor("maskc", [128, NT, S], BF16, kind="ExternalInput")
    cntw = nc.dram_tensor("cntw", [128, NT], F32, kind="ExternalInput")
    qw = nc.dram_tensor("qw", [128, NT, E], BF16, kind="ExternalInput")
    wpat = nc.dram_tensor("wpat", [128, NJ, S], BF16, kind="ExternalInput")
    ohs_seg = nc.dram_tensor("ohs_seg", [128, NT, 256], FP8, kind="ExternalInput")
    ohs_gat = nc.dram_tensor("ohs_gat", [128, NT, 256], FP8, kind="ExternalInput")
    r_out = nc.dram_tensor("r_out", [128, NT], F32, kind="ExternalOutput")

    fast = (alpha == 1.0) and (beta == 1.0) and (gamma == 1.0)
    AF = mybir.ActivationFunctionType

    NGR = (NT + GR - 1) // GR
    NGO = (NT + GO - 1) // GO

    with tile.TileContext(nc) as tc:
        with (
            tc.tile_pool(name="const", bufs=1) as constp,
            tc.tile_pool(name="rpool", bufs=2) as rpool,
            tc.tile_pool(name="zpool", bufs=2) as zpool,
            tc.tile_pool(name="small", bufs=8) as small,
            tc.tile_pool(name="xpool", bufs=4) as xpool,
            tc.tile_pool(name="ohpool", bufs=2) as ohpool,
            tc.tile_pool(name="ohgpool", bufs=2) as ohgpool,
            tc.tile_pool(name="banks", bufs=1) as bankp,
            tc.tile_pool(name="psum_z", bufs=2, space="PSUM") as pz,
            tc.tile_pool(name="psum_seg", bufs=3, space="PSUM") as pseg,
            tc.tile_pool(name="psum_gat", bufs=3, space="PSUM") as pgat,
        ):
            wpat_sb = constp.tile([128, NJ, S], BF16)
            nc.sync.dma_start(wpat_sb[:], wpat[:, :, :])
            mask_sb = constp.tile([128, NT, S], BF16)
            mask_src = maskc if fast else maskw
            qw_sb = constp.tile([128, NT, E], BF16)
            if not fast:
                cnt_sb = constp.tile([128, NT], F32)
                nc.sync.dma_start(cnt_sb[:], cntw[:, :])
            den_sb = constp.tile([128, NT], F32)
            wt_sb = constp.tile([128, NT], F32)
            rn_sb = constp.tile([128, NT], F32)
            r_sb = constp.tile([128, NT], F32)
            bank_sb = bankp.tile([128, NT, 129], BF16)

            r_groups = {}
            oh_groups = {}
            bank_ps = [None] * NT

            # ---- phase A+B: Z (PE) -> wt -> X -> segment matmuls ----
            for t in range(NT):
                g, gi = divmod(t, GR)
                if gi == 0:
                    gn = min(GR, NT - t)
                    rg = rpool.tile([128, GR, NJ, 128], R_DT)
                    nc.sync.dma_start(rg[:, 0:gn, :, :], RT[:, t:t + gn, :, :])
                    r_groups[g] = rg
                og, ogi = divmod(t, GO)
                if ogi == 0:
                    ogn = min(GO, NT - t)
                    osg = ohpool.tile([128, GO, 256], FP8)
                    nc.sync.dma_start(osg[:, 0:ogn, :], ohs_seg[:, t:t + ogn, :])
                    oh_groups[og] = osg
                    nc.sync.dma_start(
                        mask_sb[:, t:t + ogn, :], mask_src[:, t:t + ogn, :]
                    )
                    nc.sync.dma_start(
                        qw_sb[:, t:t + ogn, :], qw[:, t:t + ogn, :]
                    )

                zps = pz.tile([128, S], F32)
                rg = r_groups[g]
                for j in range(NJ):
                    nc.tensor.matmul(
                        zps[:], rg[:, gi, j, :], wpat_sb[:, j, :],
                        start=(j == 0), stop=(j == NJ - 1),
                    )

                wt_col = wt_sb[:, t:t + 1]
                if fast:
                    # wt = sum_s (max(z, tau) * mask*cnt), one fused DVE op
                    wp = zpool.tile([128, S], BF16)
                    nc.vector.scalar_tensor_tensor(
                        wp[:], zps[:], TAU, mask_sb[:, t, :],
                        op0=mybir.AluOpType.max, op1=mybir.AluOpType.mult,
                        accum_out=wt_col,
                    )
                else:
                    z = zpool.tile([128, S], F32, tag="zf32")
                    nc.vector.tensor_scalar_max(z[:], zps[:], TAU)
                    # z <- exp(alpha * ln z)   (z >= TAU > 0)
                    nc.scalar.activation(z[:], z[:], AF.Log)
                    nc.scalar.activation(z[:], z[:], AF.Exp, scale=float(alpha))
                    wp = zpool.tile([128, S], F32, tag="wpf32")
                    nc.vector.tensor_mul(wp[:], z[:], mask_sb[:, t, :])
                    a_col = small.tile([128, 1], F32)
                    nc.vector.tensor_reduce(
                        a_col[:], wp[:], axis=mybir.AxisListType.X,
                        op=mybir.AluOpType.add,
                    )
                    # wt = (A^(1/alpha) * cnt^beta)^gamma
                    #    = exp(gamma*(ln(A)/alpha + beta*ln(cnt)))
                    la = small.tile([128, 1], F32)
                    nc.scalar.activation(la[:], a_col[:], AF.Log)
                    lc = small.tile([128, 1], F32)
                    nc.scalar.activation(lc[:], cnt_sb[:, t:t + 1], AF.Log)
                    nc.vector.scalar_tensor_tensor(
                        la[:], lc[:], float(alpha * beta), la[:],
                        op0=mybir.AluOpType.mult, op1=mybir.AluOpType.add,
                    )
                    nc.scalar.activation(
                        wt_col, la[:], AF.Exp, scale=float(gamma / alpha)
                    )

                # X_t = [wt*q | wt]
                xt = xpool.tile([128, 129], BF16)
                nc.vector.tensor_scalar_mul(xt[:, 0:E], qw_sb[:, t, :], wt_col)
                nc.vector.tensor_scalar_add(xt[:, E:E + 1], wt_col, 0.0)

                oh2 = oh_groups[og]
                # leftovers of this tile into previous tile's bank (closes it)
                if t >= 1:
                    nc.tensor.matmul(
                        bank_ps[t - 1][:], oh2[:, ogi, 0:128], xt[:],
                        start=False, stop=True,
                    )
                    nc.scalar.copy(bank_sb[:, t - 1, :], bank_ps[t - 1][:])
                ps = pseg.tile([128, 129], F32)
                bank_ps[t] = ps
                last = t == NT - 1
                nc.tensor.matmul(
                    ps[:], oh2[:, ogi, 128:256], xt[:], start=True, stop=last
                )
                if last:
                    nc.scalar.copy(bank_sb[:, t, :], ps[:])

            # ---- phase C: gather num/den per row, dot with q ----
            for t in range(NT):
                og, ogi = divmod(t, GO)
                if ogi == 0:
                    ogn = min(GO, NT - t)
                    ogt = ohgpool.tile([128, GO, 256], FP8)
                    nc.sync.dma_start(ogt[:, 0:ogn, :], ohs_gat[:, t:t + ogn, :])
                    oh_groups[("g", og)] = ogt
                g2 = oh_groups[("g", og)]
                gp = pgat.tile([128, 129], F32)
                if t >= 1:
                    nc.tensor.matmul(
                        gp[:], g2[:, ogi, 0:128], bank_sb[:, t - 1, :],
                        start=True, stop=False,
                    )
                    nc.tensor.matmul(
                        gp[:], g2[:, ogi, 128:256], bank_sb[:, t, :],
                        start=False, stop=True,
                    )
                else:
                    nc.tensor.matmul(
                        gp[:], g2[:, ogi, 128:256], bank_sb[:, t, :],
                        start=True, stop=True,
                    )
                nc.vector.tensor_scalar_add(
                    den_sb[:, t:t + 1], gp[:, E:E + 1], 0.0
                )
                # rnum = sum_e num[user] * q, fused multiply+accumulate
                pq = zpool.tile([128, E], BF16, tag="pq")
                nc.vector.scalar_tensor_tensor(
                    pq[:], gp[:, 0:E], 0.0, qw_sb[:, t, :],
                    op0=mybir.AluOpType.add, op1=mybir.AluOpType.mult,
                    accum_out=rn_sb[:, t:t + 1],
                )

            # r = rnum / den, one divide for all tiles
            rec_all = small.tile([128, NT], F32, tag="rec")
            nc.vector.reciprocal(rec_all[:], den_sb[:])
            nc.vector.tensor_mul(r_sb[:], rn_sb[:], rec_all[:])

            nc.sync.dma_start(r_out[:, :], r_sb[:])

    nc.compile()
    return nc


# ----------------------------------------------------------------------------
# entry point
# ----------------------------------------------------------------------------

def kernel(users, items, R_ui, mask, w, item_emb, alpha, beta, gamma,
           _return_extras=False, _trace=False):
    users = np.asarray(users, np.int64)
    items = np.asarray(items, np.int64)
    R_ui = np.asarray(R_ui, np.float32)
    mask_b = np.asarray(mask)
    mask_f = mask_b.astype(np.float32)
    w = np.asarray(w, np.float32)
    item_emb = np.asarray(item_emb, np.float32)
    al = float(np.asarray(alpha).reshape(-1)[0])
    be = float(np.asarray(beta).reshape(-1)[0])
    ga = float(np.asarray(gamma).reshape(-1)[0])

    import time as _time

    t0 = _time.perf_counter()
    in_maps, metas, NT = _preprocess(users, items, R_ui, mask_f, w, item_emb)
    t1 = _time.perf_counter()
    nc = build_program(NT, al, be, ga)
    t2 = _time.perf_counter()
    res = run_bass_kernel_spmd(
        nc, in_maps, core_ids=list(range(N_CORES)), trace=_trace
    )
    t3 = _time.perf_counter()
    print(
        f"[kernel] preprocess {t1-t0:.1f}s  build+schedule {t2-t1:.1f}s  "
        f"compile+run {t3-t2:.1f}s"
    )

    n = users.shape[0]
    r = np.empty(n, np.float32)
    for c in range(N_CORES):
        p, nc_rows = metas[c]
        shard = res.results[c]["r_out"].T.reshape(-1)[:nc_rows]
        r[p] = shard
    if _return_extras:
        return r, res
    return r


# revision 9
# speedup vs baseline: 5.5079x; 1.3047x over previous
"""CDMF segment-reduce kernel for 8 Trainium2 NeuronCores.

Strategy
--------
Host (cheap, index-only + one big gather):
  * stable-sort rows by user id; cut the 100k rows into 8 shards at user
    boundaries ("expert-style sharding of user segments") so each core owns a
    disjoint user range -> no cross-core reduction needed at all.
  * pad every shard to NT*128 rows (mask=0 rows contribute exactly 0).
  * compact each row's valid (mask=1) seq-slices to the front; masked slices
    are never used by the math, so they are not transferred at all. Per-tile
    chunk counts CH[t] (shared across cores) bound the compacted width.
  * pre-gather q = item_emb[items] per shard; R in fp8e4m3, rest bf16.
  * transpose R into PE-friendly chunks RT[(c%2)*64+d, tile, j, row] so the
    feature contraction runs on the tensor engine.
  * build per-tile one-hot matrices (fp8, values 0/1 exact) mapping the 128
    rows of a tile to user-slots of a PSUM "bank" (bank b = users first seen
    in tile b). Only SLOTS (max users/tile, rounded up) slots are kept.

Device (one SPMD program on 8 cores):
  * PE: per tile, CH[t] accumulating matmuls lhsT=RT-chunk [K=128=(2 slices
    x 64 d), M=128 rows], rhs=w-pattern [128, W] -> Z [128 rows, W] in PSUM.
  * DVE: one fused scalar_tensor_tensor (max(Z,tau) * maskc, accum) ->
    per-row weight wt; two tensor_scalar ops build X = [wt*q | wt].
  * PE one-hot matmuls accumulate per-user [sum wt*q | sum wt] (N=129) into
    PSUM banks; ACT flushes each bank to SBUF (bf16).
  * transposed one-hot matmuls gather num[user]/den[user] back per row;
    fused scalar_tensor_tensor computes sum_e num*q; reciprocal+mul -> r.
"""

import numpy as np
import ml_dtypes

import concourse.bass as bass
import concourse.tile as tile
from concourse import bacc, mybir
from concourse.bass_utils import run_bass_kernel_spmd

N_CORES = 8
TAU = 0.01
S = 50          # seq_len
D = 64          # n_features
E = 128         # emb_dim
NJ = S // 2     # max PE k-chunks per tile (2 s-slices of 64 features each)
GO = 8          # one-hot / mask / q tiles per DMA
F32 = mybir.dt.float32
BF16 = mybir.dt.bfloat16
FP8 = mybir.dt.float8e4
R_DT = FP8      # R stream dtype: fp8e4m3 keeps max-normalized err ~8e-3

NP_BF16 = ml_dtypes.bfloat16
NP_FP8 = mybir.dt.np(FP8)
NP_R = mybir.dt.np(R_DT)

# R-tile DMA group plan: small groups first to shorten the pipeline prologue
def _group_plan(NT):
    plan = [1, 1, 2]
    done = sum(plan)
    if done >= NT:
        plan, acc = [], 0
        for g in [1, 1, 2]:
            if acc >= NT:
                break
            plan.append(min(g, NT - acc))
            acc += plan[-1]
        return plan
    while done < NT:
        g = min(4, NT - done)
        plan.append(g)
        done += g
    return plan


# Data-dependent build parameters (set by _preprocess, read by
# build_program's defaults so `build_program(NT)` builds the same program
# that kernel() runs).
_BUILD_PARAMS = None


# ----------------------------------------------------------------------------
# host-side preprocessing
# ----------------------------------------------------------------------------

def _preprocess(users, items, R_ui, mask, w, item_emb):
    global _BUILD_PARAMS
    n = users.shape[0]
    perm = np.argsort(users, kind="stable")
    users_s = users[perm]

    # shard cuts at user boundaries
    cuts = [0]
    for c in range(1, N_CORES):
        t = round(c * n / N_CORES)
        while 0 < t < n and users_s[t] == users_s[t - 1]:
            t += 1
        cuts.append(min(t, n))
    cuts.append(n)
    sizes = [cuts[c + 1] - cuts[c] for c in range(N_CORES)]
    NT = max(1, int(np.ceil(max(sizes) / 128)))
    NPAD = NT * 128

    q_full = item_emb[items]  # [n, E]
    w_bf = np.asarray(w, NP_BF16)

    # ---- pass 1: per-core masks -> shared chunk counts + slot count ----
    mks = []
    cnts = []
    slots_needed = 1
    metas = []
    row_maps = []
    for c in range(N_CORES):
        lo, hi = cuts[c], cuts[c + 1]
        nc_rows = hi - lo
        p = perm[lo:hi]
        mk = np.zeros((NPAD, S), np.float32)
        mk[:nc_rows] = mask[p]
        mks.append(mk)
        cnts.append(mk.sum(1).astype(np.int64))
        metas.append((p, nc_rows))

        u = np.empty(NPAD, np.int64)
        u[:nc_rows] = users_s[lo:hi]
        u[nc_rows:] = u[nc_rows - 1] if nc_rows > 0 else 0
        uniq, first_idx, inv = np.unique(u, return_index=True,
                                         return_inverse=True)
        ft = first_idx // 128
        gstart = np.searchsorted(ft, ft, side="left")
        slot = np.arange(len(uniq)) - gstart
        slots_needed = max(slots_needed, int(np.bincount(ft).max()))
        row_maps.append((ft[inv], slot[inv]))

    cnt_mat = np.stack(cnts)                        # [cores, NPAD]
    tile_max = cnt_mat.reshape(N_CORES, NT, 128).max(-1).max(0)  # [NT]
    CH = np.maximum(1, (tile_max + 1) // 2).astype(np.int64)     # chunks/tile
    OFF = np.concatenate([[0], np.cumsum(CH)])       # chunk offsets
    TOT = int(OFF[-1])
    Wt = 2 * CH                                      # Z columns per tile
    WOFF = np.concatenate([[0], np.cumsum(Wt)])
    WTOT = int(WOFF[-1])
    SLOTS = int(min(128, ((slots_needed + 31) // 32) * 32))
    assert slots_needed <= 128, "bank overflow"

    _BUILD_PARAMS = {
        "NT": NT,
        "CH": tuple(int(x) for x in CH),
        "SLOTS": SLOTS,
    }

    # ---- pass 2: per-core arrays ----
    in_maps = []
    for c in range(N_CORES):
        p, nc_rows = metas[c]
        mk = mks[c]
        cnt = cnts[c]

        # compact valid slices to the front of each row (stable order)
        Rp = np.zeros((NPAD, S, D), NP_R)
        Rp[:nc_rows] = R_ui[p]
        vidx = np.argsort(mk <= 0, axis=1, kind="stable")  # valid first
        cmp = np.take_along_axis(Rp, vidx[:, :, None], axis=1)

        RT = np.empty((128, TOT * 128), NP_R)
        for t in range(NT):
            ch = int(CH[t])
            blk = cmp[t * 128:(t + 1) * 128, 0:2 * ch, :]
            RT[:, OFF[t] * 128:OFF[t + 1] * 128] = (
                blk.reshape(128, ch, 2, D).transpose(2, 3, 1, 0)
                .reshape(128, ch * 128)
            )

        # compacted masks: column c of tile t is live iff c < cnt[row]
        cntw = np.ascontiguousarray(
            cnt.reshape(NT, 128).T, np.float32)      # [128, NT]
        maskcF = np.zeros((128, WTOT), NP_BF16)
        maskwF = np.zeros((128, WTOT), NP_BF16)
        for t in range(NT):
            crow = cnt[t * 128:(t + 1) * 128]        # [128]
            live = np.arange(2 * int(CH[t]))[None, :] < crow[:, None]
            maskwF[:, WOFF[t]:WOFF[t + 1]] = live
            maskcF[:, WOFF[t]:WOFF[t + 1]] = live * crow[:, None]

        qp = np.zeros((NPAD, E), NP_BF16)
        qp[:nc_rows] = q_full[p]
        qw = np.ascontiguousarray(qp.reshape(NT, 128, E).transpose(1, 0, 2))

        # w-pattern for the PE feature contraction: [128=(c%2)*64+d, NJ, S]
        wpat = np.zeros((128, NJ, S), NP_BF16)
        for sp in range(2):
            for j in range(NJ):
                wpat[sp * 64:(sp + 1) * 64, j, 2 * j + sp] = w_bf

        fr, sr = row_maps[c]
        ii = np.arange(NPAD)
        tt, kk = ii // 128, ii % 128
        own = fr == tt
        prev = fr == tt - 1
        assert np.all(own | prev), "user spans >2 tiles (unexpected padding)"

        seg = np.zeros((128, NT, 2 * SLOTS), NP_FP8)
        gat = np.zeros((SLOTS, NT, 256), NP_FP8)
        seg[kk[own], tt[own], SLOTS + sr[own]] = 1.0
        seg[kk[prev], tt[prev], sr[prev]] = 1.0
        gat[sr[own], tt[own], 128 + kk[own]] = 1.0
        gat[sr[prev], tt[prev], kk[prev]] = 1.0

        in_maps.append(
            {
                "RT": RT,
                "maskw": maskwF,
                "maskc": maskcF,
                "cntw": cntw,
                "qw": qw,
                "wpat": wpat,
                "ohs_seg": seg,
                "ohs_gat": gat,
            }
        )
    return in_maps, metas, NT


# ----------------------------------------------------------------------------
# device program
# ----------------------------------------------------------------------------

def build_program(NT, alpha=1.0, beta=1.0, gamma=1.0, params=None):
    if params is None:
        params = _BUILD_PARAMS
    if params is None or params["NT"] != NT:
        params = {"NT": NT, "CH": (NJ,) * NT, "SLOTS": 128}
    CH = params["CH"]
    SLOTS = params["SLOTS"]
    OFF = [0]
    for ch in CH:
        OFF.append(OFF[-1] + ch)
    TOT = OFF[-1]
    WOFF = [2 * o for o in OFF]
    WTOT = 2 * TOT

    nc = bacc.Bacc(
        "TRN2", target_bir_lowering=False, debug=False, num_devices=N_CORES
    )

    RT = nc.dram_tensor("RT", [128, TOT * 128], R_DT, kind="ExternalInput")
    maskw = nc.dram_tensor("maskw", [128, WTOT], BF16, kind="ExternalInput")
    maskc = nc.dram_tensor("maskc", [128, WTOT], BF16, kind="ExternalInput")
    cntw = nc.dram_tensor("cntw", [128, NT], F32, kind="ExternalInput")
    qw = nc.dram_tensor("qw", [128, NT, E], BF16, kind="ExternalInput")
    wpat = nc.dram_tensor("wpat", [128, NJ, S], BF16, kind="ExternalInput")
    ohs_seg = nc.dram_tensor("ohs_seg", [128, NT, 2 * SLOTS], FP8,
                             kind="ExternalInput")
    ohs_gat = nc.dram_tensor("ohs_gat", [SLOTS, NT, 256], FP8,
                             kind="ExternalInput")
    r_out = nc.dram_tensor("r_out", [128, NT], F32, kind="ExternalOutput")

    fast = (alpha == 1.0) and (beta == 1.0) and (gamma == 1.0)
    AF = mybir.ActivationFunctionType

    plan = _group_plan(NT)
    gstart = [0]
    for g in plan:
        gstart.append(gstart[-1] + g)
    MAXC = max(OFF[gstart[i + 1]] - OFF[gstart[i]] for i in range(len(plan)))

    with tile.TileContext(nc) as tc:
        with (
            tc.tile_pool(name="const", bufs=1) as constp,
            tc.tile_pool(name="rpool", bufs=2) as rpool,
            tc.tile_pool(name="zpool", bufs=2) as zpool,
            tc.tile_pool(name="small", bufs=8) as small,
            tc.tile_pool(name="xpool", bufs=4) as xpool,
            tc.tile_pool(name="ohpool", bufs=2) as ohpool,
            tc.tile_pool(name="ohgpool", bufs=2) as ohgpool,
            tc.tile_pool(name="banks", bufs=1) as bankp,
            tc.tile_pool(name="psum_z", bufs=2, space="PSUM") as pz,
            tc.tile_pool(name="psum_seg", bufs=3, space="PSUM") as pseg,
            tc.tile_pool(name="psum_gat", bufs=3, space="PSUM") as pgat,
        ):
            wpat_sb = constp.tile([128, NJ, S], BF16)
            nc.sync.dma_start(wpat_sb[:], wpat[:, :, :])
            mask_sb = constp.tile([128, WTOT], BF16)
            mask_src = maskc if fast else maskw
            qw_sb = constp.tile([128, NT, E], BF16)
            if not fast:
                cnt_sb = constp.tile([128, NT], F32)
                nc.sync.dma_start(cnt_sb[:], cntw[:, :])
            den_sb = constp.tile([128, NT], F32)
            wt_sb = constp.tile([128, NT], F32)
            rn_sb = constp.tile([128, NT], F32)
            r_sb = constp.tile([128, NT], F32)
            bank_sb = bankp.tile([128, NT, 129], BF16)

            r_groups = {}
            oh_groups = {}
            bank_ps = [None] * NT
            tile_group = []
            for gi_, g in enumerate(plan):
                tile_group += [gi_] * g

            # ---- phase A+B: Z (PE) -> wt -> X -> segment matmuls ----
            for t in range(NT):
                g = tile_group[t]
                if t == gstart[g]:
                    t1 = gstart[g + 1]
                    c0, c1 = OFF[t], OFF[t1]
                    rg = rpool.tile([128, MAXC * 128], R_DT)
                    nc.sync.dma_start(
                        rg[:, 0:(c1 - c0) * 128], RT[:, c0 * 128:c1 * 128]
                    )
                    r_groups[g] = rg
                og, ogi = divmod(t, GO)
                if ogi == 0:
                    ogn = min(GO, NT - t)
                    osg = ohpool.tile([128, GO, 2 * SLOTS], FP8)
                    nc.sync.dma_start(
                        osg[:, 0:ogn, :], ohs_seg[:, t:t + ogn, :]
                    )
                    oh_groups[og] = osg
                    nc.sync.dma_start(
                        mask_sb[:, WOFF[t]:WOFF[t + ogn]],
                        mask_src[:, WOFF[t]:WOFF[t + ogn]],
                    )
                    nc.sync.dma_start(
                        qw_sb[:, t:t + ogn, :], qw[:, t:t + ogn, :]
                    )

                W = 2 * CH[t]
                rg = r_groups[g]
                base = (OFF[t] - OFF[gstart[g]]) * 128
                zps = pz.tile([128, S], F32)
                for j in range(CH[t]):
                    nc.tensor.matmul(
                        zps[:, 0:W], rg[:, base + j * 128:base + (j + 1) * 128],
                        wpat_sb[:, j, 0:W],
                        start=(j == 0), stop=(j == CH[t] - 1),
                    )

                wt_col = wt_sb[:, t:t + 1]
                mcol = mask_sb[:, WOFF[t]:WOFF[t] + W]
                if fast:
                    # wt = sum_c (max(z, tau) * mask*cnt), one fused DVE op
                    wp = zpool.tile([128, S], BF16)
                    nc.vector.scalar_tensor_tensor(
                        wp[:, 0:W], zps[:, 0:W], TAU, mcol,
                        op0=mybir.AluOpType.max, op1=mybir.AluOpType.mult,
                        accum_out=wt_col,
                    )
                else:
                    z = zpool.tile([128, S], F32, tag="zf32")
                    nc.vector.tensor_scalar_max(z[:, 0:W], zps[:, 0:W], TAU)
                    # z <- exp(alpha * ln z)   (z >= TAU > 0)
                    nc.scalar.activation(z[:, 0:W], z[:, 0:W], AF.Log)
                    nc.scalar.activation(z[:, 0:W], z[:, 0:W], AF.Exp,
                                         scale=float(alpha))
                    wp = zpool.tile([128, S], F32, tag="wpf32")
                    nc.vector.tensor_mul(wp[:, 0:W], z[:, 0:W], mcol)
                    a_col = small.tile([128, 1], F32)
                    nc.vector.tensor_reduce(
                        a_col[:], wp[:, 0:W], axis=mybir.AxisListType.X,
                        op=mybir.AluOpType.add,
                    )
                    # wt = (A^(1/alpha) * cnt^beta)^gamma
                    #    = exp(gamma*(ln(A)/alpha + beta*ln(cnt)))
                    la = small.tile([128, 1], F32)
                    nc.scalar.activation(la[:], a_col[:], AF.Log)
                    lc = small.tile([128, 1], F32)
                    nc.scalar.activation(lc[:], cnt_sb[:, t:t + 1], AF.Log)
                    nc.vector.scalar_tensor_tensor(
                        la[:], lc[:], float(alpha * beta), la[:],
                        op0=mybir.AluOpType.mult, op1=mybir.AluOpType.add,
                    )
                    nc.scalar.activation(
                        wt_col, la[:], AF.Exp, scale=float(gamma / alpha)
                    )

                # X_t = [wt*q | wt]
                xt = xpool.tile([128, 129], BF16)
                nc.vector.tensor_scalar_mul(xt[:, 0:E], qw_sb[:, t, :], wt_col)
                nc.vector.tensor_scalar_add(xt[:, E:E + 1], wt_col, 0.0)

                oh2 = oh_groups[og]
                # leftovers of this tile into previous tile's bank (closes it)
                if t >= 1:
                    nc.tensor.matmul(
                        bank_ps[t - 1][0:SLOTS, :], oh2[:, ogi, 0:SLOTS],
                        xt[:], start=False, stop=True,
                    )
                    nc.scalar.copy(
                        bank_sb[0:SLOTS, t - 1, :], bank_ps[t - 1][0:SLOTS, :]
                    )
                ps = pseg.tile([128, 129], F32)
                bank_ps[t] = ps
                last = t == NT - 1
                nc.tensor.matmul(
                    ps[0:SLOTS, :], oh2[:, ogi, SLOTS:2 * SLOTS], xt[:],
                    start=True, stop=last,
                )
                if last:
                    nc.scalar.copy(bank_sb[0:SLOTS, t, :], ps[0:SLOTS, :])

            # ---- phase C: gather num/den per row, dot with q ----
            for t in range(NT):
                og, ogi = divmod(t, GO)
                if ogi == 0:
                    ogn = min(GO, NT - t)
                    ogt = ohgpool.tile([128, GO, 256], FP8)
                    nc.sync.dma_start(
                        ogt[0:SLOTS, 0:ogn, :], ohs_gat[:, t:t + ogn, :]
                    )
                    oh_groups[("g", og)] = ogt
                g2 = oh_groups[("g", og)]
                gp = pgat.tile([128, 129], F32)
                if t >= 1:
                    nc.tensor.matmul(
                        gp[:], g2[0:SLOTS, ogi, 0:128],
                        bank_sb[0:SLOTS, t - 1, :],
                        start=True, stop=False,
                    )
                    nc.tensor.matmul(
                        gp[:], g2[0:SLOTS, ogi, 128:256],
                        bank_sb[0:SLOTS, t, :],
                        start=False, stop=True,
                    )
                else:
                    nc.tensor.matmul(
                        gp[:], g2[0:SLOTS, ogi, 128:256],
                        bank_sb[0:SLOTS, t, :],
                        start=True, stop=True,
                    )
                nc.scalar.copy(den_sb[:, t:t + 1], gp[:, E:E + 1])
                # rnum = sum_e num[user] * q, fused multiply+accumulate
                pq = zpool.tile([128, E], BF16, tag="pq")
                nc.vector.scalar_tensor_tensor(
                    pq[:], gp[:, 0:E], 0.0, qw_sb[:, t, :],
                    op0=mybir.AluOpType.add, op1=mybir.AluOpType.mult,
                    accum_out=rn_sb[:, t:t + 1],
                )

            # r = rnum / den, one divide for all tiles
            rec_all = small.tile([128, NT], F32, tag="rec")
            nc.vector.reciprocal(rec_all[:], den_sb[:])
            nc.vector.tensor_mul(r_sb[:], rn_sb[:], rec_all[:])

            nc.sync.dma_start(r_out[:, :], r_sb[:])

    nc.compile()
    return nc


# ----------------------------------------------------------------------------
# entry point
# ----------------------------------------------------------------------------

def kernel(users, items, R_ui, mask, w, item_emb, alpha, beta, gamma,
           _return_extras=False, _trace=False):
    users = np.asarray(users, np.int64)
    items = np.asarray(items, np.int64)
    R_ui = np.asarray(R_ui, np.float32)
    mask_b = np.asarray(mask)
    mask_f = mask_b.astype(np.float32)
    w = np.asarray(w, np.float32)
    item_emb = np.asarray(item_emb, np.float32)
    al = float(np.asarray(alpha).reshape(-1)[0])
    be = float(np.asarray(beta).reshape(-1)[0])
    ga = float(np.asarray(gamma).reshape(-1)[0])

    import time as _time

    t0 = _time.perf_counter()
    in_maps, metas, NT = _preprocess(users, items, R_ui, mask_f, w, item_emb)
    t1 = _time.perf_counter()
    nc = build_program(NT, al, be, ga)
    t2 = _time.perf_counter()
    res = run_bass_kernel_spmd(
        nc, in_maps, core_ids=list(range(N_CORES)), trace=_trace
    )
    t3 = _time.perf_counter()
    print(
        f"[kernel] preprocess {t1-t0:.1f}s  build+schedule {t2-t1:.1f}s  "
        f"compile+run {t3-t2:.1f}s"
    )

    n = users.shape[0]
    r = np.empty(n, np.float32)
    for c in range(N_CORES):
        p, nc_rows = metas[c]
        shard = res.results[c]["r_out"].T.reshape(-1)[:nc_rows]
        r[p] = shard
    if _return_extras:
        return r, res
    return r


# revision 26
# speedup vs baseline: 6.6790x; 1.2126x over previous
"""CDMF segment-reduce kernel for 8 Trainium2 NeuronCores.

Strategy
--------
Host (cheap, index-only + one big gather):
  * stable-sort rows by user id; cut the 100k rows into 8 shards at user
    boundaries ("expert-style sharding of user segments") so each core owns a
    disjoint user range -> no cross-core reduction needed at all.
  * pad every shard to NT*128 rows (mask=0 rows contribute exactly 0).
  * compact each row's valid (mask=1) seq-slices to the front; masked slices
    are never used by the math, so they are not transferred at all. Per-tile
    chunk counts CH[t] (shared across cores) bound the compacted width.
  * pre-gather q = item_emb[items] per shard; R in fp8e4m3, rest bf16.
  * transpose R into PE-friendly chunks RT[(c%2)*64+d, tile, j, row] so the
    feature contraction runs on the tensor engine.
  * build per-tile one-hot matrices (fp8, values 0/1 exact) mapping the 128
    rows of a tile to user-slots of a PSUM "bank" (bank b = users first seen
    in tile b). Only SLOTS (max users/tile, rounded up) slots are kept.

Device (one SPMD program on 8 cores):
  * PE: per tile, CH[t] accumulating matmuls lhsT=RT-chunk [K=128=(2 slices
    x 64 d), M=128 rows], rhs=w-pattern [128, W] -> Z [128 rows, W] in PSUM.
  * DVE: one fused scalar_tensor_tensor (max(Z,tau) * maskc, accum) ->
    per-row weight wt; two tensor_scalar ops build X = [wt*q | wt].
  * PE one-hot matmuls accumulate per-user [sum wt*q | sum wt] (N=129) into
    PSUM banks; ACT flushes each bank to SBUF (bf16).
  * transposed one-hot matmuls gather num[user]/den[user] back per row;
    fused scalar_tensor_tensor computes sum_e num*q; reciprocal+mul -> r.
"""

import numpy as np
import ml_dtypes

import concourse.bass as bass
import concourse.tile as tile
from concourse import bacc, mybir
from concourse.bass_utils import run_bass_kernel_spmd

N_CORES = 8
TAU = 0.01
S = 50          # seq_len
D = 64          # n_features
E = 128         # emb_dim
NJ = S // 2     # max PE k-chunks per tile (2 s-slices of 64 features each)
GO = 8          # one-hot / mask / q tiles per DMA
F32 = mybir.dt.float32
BF16 = mybir.dt.bfloat16
FP8 = mybir.dt.float8e4
# R stream dtype: fp8 e3m4 with an exact 2x pre-scale (2 folded into R,
# 1/2 into w) -- 4 mantissa bits halve the Z error vs e4m3 at equal bytes.
R_DT = mybir.dt.float8e3
R_SCALE = 2.0
R_CLIP = 15.5 / R_SCALE

NP_BF16 = ml_dtypes.bfloat16
NP_FP8 = mybir.dt.np(FP8)
NP_R = mybir.dt.np(R_DT)


# R-tile DMA group plan: small groups at both ends to shorten the pipeline
# prologue (first compute waits on a small DMA) and epilogue (last tile's
# chain starts as early as possible).
def _group_plan(NT):
    front, tail = [1, 1, 2], [2, 1]
    if NT <= sum(front) + sum(tail):
        plan, acc = [], 0
        for g in front + tail:
            if acc >= NT:
                break
            plan.append(min(g, NT - acc))
            acc += plan[-1]
        return plan
    mid = NT - sum(front) - sum(tail)
    plan = front + [4] * (mid // 4)
    if mid % 4:
        plan.append(mid % 4)
    return plan + tail


# Data-dependent build parameters (set by _preprocess, read by
# build_program's defaults so `build_program(NT)` builds the same program
# that kernel() runs).
_BUILD_PARAMS = None


# ----------------------------------------------------------------------------
# host-side preprocessing
# ----------------------------------------------------------------------------

def _preprocess(users, items, R_ui, mask, w, item_emb):
    global _BUILD_PARAMS
    n = users.shape[0]
    perm = np.argsort(users, kind="stable")
    users_s = users[perm]

    # shard cuts at user boundaries
    cuts = [0]
    for c in range(1, N_CORES):
        t = round(c * n / N_CORES)
        while 0 < t < n and users_s[t] == users_s[t - 1]:
            t += 1
        cuts.append(min(t, n))
    cuts.append(n)
    sizes = [cuts[c + 1] - cuts[c] for c in range(N_CORES)]
    NT = max(1, int(np.ceil(max(sizes) / 128)))
    NPAD = NT * 128

    q_full = item_emb[items]  # [n, E]
    w_bf = np.asarray(w, NP_BF16)

    # ---- pass 1: per-core masks -> shared chunk counts + slot count ----
    mks = []
    cnts = []
    slots_needed = 1
    metas = []
    row_maps = []
    for c in range(N_CORES):
        lo, hi = cuts[c], cuts[c + 1]
        nc_rows = hi - lo
        p = perm[lo:hi]
        mk = np.zeros((NPAD, S), np.float32)
        mk[:nc_rows] = mask[p]
        mks.append(mk)
        cnts.append(mk.sum(1).astype(np.int64))
        metas.append((p, nc_rows))

        u = np.empty(NPAD, np.int64)
        u[:nc_rows] = users_s[lo:hi]
        u[nc_rows:] = u[nc_rows - 1] if nc_rows > 0 else 0
        uniq, first_idx, inv = np.unique(u, return_index=True,
                                         return_inverse=True)
        ft = first_idx // 128
        gstart = np.searchsorted(ft, ft, side="left")
        slot = np.arange(len(uniq)) - gstart
        slots_needed = max(slots_needed, int(np.bincount(ft).max()))
        row_maps.append((ft[inv], slot[inv]))

    cnt_mat = np.stack(cnts)                        # [cores, NPAD]
    tile_max = cnt_mat.reshape(N_CORES, NT, 128).max(-1).max(0)  # [NT]
    CH = np.maximum(1, (tile_max + 1) // 2).astype(np.int64)     # chunks/tile
    OFF = np.concatenate([[0], np.cumsum(CH)])       # chunk offsets
    TOT = int(OFF[-1])
    Wt = 2 * CH                                      # Z columns per tile
    WOFF = np.concatenate([[0], np.cumsum(Wt)])
    WTOT = int(WOFF[-1])
    SLOTS = int(min(128, ((slots_needed + 31) // 32) * 32))
    assert slots_needed <= 128, "bank overflow"

    _BUILD_PARAMS = {
        "NT": NT,
        "CH": tuple(int(x) for x in CH),
        "SLOTS": SLOTS,
    }

    # ---- pass 2: per-core arrays ----
    in_maps = []
    for c in range(N_CORES):
        p, nc_rows = metas[c]
        mk = mks[c]
        cnt = cnts[c]

        # compact valid slices to the front of each row (stable order)
        Rp = np.zeros((NPAD, S, D), NP_R)
        Rp[:nc_rows] = np.clip(R_ui[p], -R_CLIP, R_CLIP) * R_SCALE
        vidx = np.argsort(mk <= 0, axis=1, kind="stable")  # valid first
        cmp = np.take_along_axis(Rp, vidx[:, :, None], axis=1)

        RT = np.empty((128, TOT * 128), NP_R)
        for t in range(NT):
            ch = int(CH[t])
            blk = cmp[t * 128:(t + 1) * 128, 0:2 * ch, :]
            RT[:, OFF[t] * 128:OFF[t + 1] * 128] = (
                blk.reshape(128, ch, 2, D).transpose(2, 3, 1, 0)
                .reshape(128, ch * 128)
            )

        # the compacted mask (column c of tile t live iff c < cnt[row]) is
        # built on-device from cnt via an iota compare -- only cnt ships.
        cntw = np.ascontiguousarray(
            cnt.reshape(NT, 128).T, np.float32)      # [128, NT]
        iota50 = np.ascontiguousarray(
            np.broadcast_to(np.arange(S, dtype=np.float32), (128, S))
        ).astype(NP_BF16)

        qp = np.zeros((NPAD, E), NP_BF16)
        qp[:nc_rows] = q_full[p]
        qw = np.ascontiguousarray(qp.reshape(NT, 128, E).transpose(1, 0, 2))

        # w-pattern for the PE feature contraction: [128=(c%2)*64+d, NJ, S]
        # (carries the 1/R_SCALE compensation -- exact, power of two)
        wpat = np.zeros((128, NJ, S), NP_BF16)
        wb = (w_bf.astype(np.float32) / R_SCALE).astype(NP_BF16)
        for sp in range(2):
            for j in range(NJ):
                wpat[sp * 64:(sp + 1) * 64, j, 2 * j + sp] = wb

        fr, sr = row_maps[c]
        ii = np.arange(NPAD)
        tt, kk = ii // 128, ii % 128
        own = fr == tt
        prev = fr == tt - 1
        assert np.all(own | prev), "user spans >2 tiles (unexpected padding)"

        seg = np.zeros((128, NT, 2 * SLOTS), NP_FP8)
        gat = np.zeros((SLOTS, NT, 256), NP_FP8)
        seg[kk[own], tt[own], SLOTS + sr[own]] = 1.0
        seg[kk[prev], tt[prev], sr[prev]] = 1.0
        gat[sr[own], tt[own], 128 + kk[own]] = 1.0
        gat[sr[prev], tt[prev], kk[prev]] = 1.0

        in_maps.append(
            {
                "RT": RT,
                "cntw": cntw,
                "iota50": iota50,
                "qw": qw,
                "wpat": wpat,
                "ohs_seg": seg,
                "ohs_gat": gat,
            }
        )
    return in_maps, metas, NT


# ----------------------------------------------------------------------------
# device program
# ----------------------------------------------------------------------------

def build_program(NT, alpha=1.0, beta=1.0, gamma=1.0, params=None):
    if params is None:
        params = _BUILD_PARAMS
    if params is None or params["NT"] != NT:
        params = {"NT": NT, "CH": (NJ,) * NT, "SLOTS": 128}
    CH = params["CH"]
    SLOTS = params["SLOTS"]
    OFF = [0]
    for ch in CH:
        OFF.append(OFF[-1] + ch)
    TOT = OFF[-1]
    WOFF = [2 * o for o in OFF]
    WTOT = 2 * TOT

    nc = bacc.Bacc(
        "TRN2", target_bir_lowering=False, debug=False, num_devices=N_CORES
    )

    RT = nc.dram_tensor("RT", [128, TOT * 128], R_DT, kind="ExternalInput")
    cntw = nc.dram_tensor("cntw", [128, NT], F32, kind="ExternalInput")
    iota50 = nc.dram_tensor("iota50", [128, S], BF16, kind="ExternalInput")
    qw = nc.dram_tensor("qw", [128, NT, E], BF16, kind="ExternalInput")
    wpat = nc.dram_tensor("wpat", [128, NJ, S], BF16, kind="ExternalInput")
    ohs_seg = nc.dram_tensor("ohs_seg", [128, NT, 2 * SLOTS], FP8,
                             kind="ExternalInput")
    ohs_gat = nc.dram_tensor("ohs_gat", [SLOTS, NT, 256], FP8,
                             kind="ExternalInput")
    r_out = nc.dram_tensor("r_out", [128, NT], F32, kind="ExternalOutput")

    fast = (alpha == 1.0) and (beta == 1.0) and (gamma == 1.0)
    AF = mybir.ActivationFunctionType

    plan = _group_plan(NT)
    gstart = [0]
    for g in plan:
        gstart.append(gstart[-1] + g)
    MAXC = max(OFF[gstart[i + 1]] - OFF[gstart[i]] for i in range(len(plan)))

    with tile.TileContext(nc) as tc:
        with (
            tc.tile_pool(name="const", bufs=1) as constp,
            tc.tile_pool(name="rpool", bufs=3) as rpool,
            tc.tile_pool(name="zpool", bufs=2) as zpool,
            tc.tile_pool(name="mpool", bufs=2) as mpool,
            tc.tile_pool(name="small", bufs=8) as small,
            tc.tile_pool(name="xpool", bufs=4) as xpool,
            tc.tile_pool(name="ohpool", bufs=3) as ohpool,
            tc.tile_pool(name="ohgpool", bufs=3) as ohgpool,
            tc.tile_pool(name="banks", bufs=1) as bankp,
            tc.tile_pool(name="psum_z", bufs=2, space="PSUM") as pz,
            tc.tile_pool(name="psum_seg", bufs=3, space="PSUM") as pseg,
            tc.tile_pool(name="psum_gat", bufs=3, space="PSUM") as pgat,
        ):
            wpat_sb = constp.tile([128, NJ, S], BF16)
            nc.sync.dma_start(wpat_sb[:], wpat[:, :, :])
            iota_sb = constp.tile([128, S], BF16)
            nc.sync.dma_start(iota_sb[:], iota50[:, :])
            qw_sb = constp.tile([128, NT, E], BF16)
            cnt_sb = constp.tile([128, NT], F32)
            nc.sync.dma_start(cnt_sb[:], cntw[:, :])
            den_sb = constp.tile([128, NT], F32)
            wt_sb = constp.tile([128, NT], F32)
            rn_sb = constp.tile([128, NT], F32)
            r_sb = constp.tile([128, NT], F32)
            bank_sb = bankp.tile([128, NT, 129], BF16)

            r_groups = {}
            oh_groups = {}
            bank_ps = [None] * NT
            tile_group = []
            for gi_, g in enumerate(plan):
                tile_group += [gi_] * g

            def phase_a(t):
                g = tile_group[t]
                if t == gstart[g]:
                    t1 = gstart[g + 1]
                    c0, c1 = OFF[t], OFF[t1]
                    rg = rpool.tile([128, MAXC * 128], R_DT)
                    nc.sync.dma_start(
                        rg[:, 0:(c1 - c0) * 128], RT[:, c0 * 128:c1 * 128]
                    )
                    r_groups[g] = rg
                og, ogi = divmod(t, GO)
                if ogi == 0:
                    ogn = min(GO, NT - t)
                    osg = ohpool.tile([128, GO, 2 * SLOTS], FP8)
                    nc.sync.dma_start(
                        osg[:, 0:ogn, :], ohs_seg[:, t:t + ogn, :]
                    )
                    oh_groups[og] = osg
                    nc.sync.dma_start(
                        qw_sb[:, t:t + ogn, :], qw[:, t:t + ogn, :]
                    )

                W = 2 * CH[t]
                rg = r_groups[g]
                base = (OFF[t] - OFF[gstart[g]]) * 128
                zps = pz.tile([128, S], F32)
                for j in range(CH[t]):
                    nc.tensor.matmul(
                        zps[:, 0:W], rg[:, base + j * 128:base + (j + 1) * 128],
                        wpat_sb[:, j, 0:W],
                        start=(j == 0), stop=(j == CH[t] - 1),
                    )

                wt_col = wt_sb[:, t:t + 1]
                cnt_col = cnt_sb[:, t:t + 1]
                # mask column c live iff c < cnt; fast path pre-scales by cnt
                mct = mpool.tile([128, S], BF16)
                mcol = mct[:, 0:W]
                nc.vector.tensor_scalar(
                    mcol, iota_sb[:, 0:W], cnt_col,
                    cnt_col if fast else 1.0,
                    op0=mybir.AluOpType.is_lt, op1=mybir.AluOpType.mult,
                )
                if fast:
                    # wt = sum_c (max(z, tau) * mask*cnt), one fused DVE op
                    wp = zpool.tile([128, S], BF16)
                    nc.vector.scalar_tensor_tensor(
                        wp[:, 0:W], zps[:, 0:W], TAU, mcol,
                        op0=mybir.AluOpType.max, op1=mybir.AluOpType.mult,
                        accum_out=wt_col,
                    )
                else:
                    z = zpool.tile([128, S], F32, tag="zf32")
                    nc.vector.tensor_scalar_max(z[:, 0:W], zps[:, 0:W], TAU)
                    # z <- exp(alpha * ln z)   (z >= TAU > 0)
                    nc.scalar.activation(z[:, 0:W], z[:, 0:W], AF.Log)
                    nc.scalar.activation(z[:, 0:W], z[:, 0:W], AF.Exp,
                                         scale=float(alpha))
                    wp = zpool.tile([128, S], F32, tag="wpf32")
                    nc.vector.tensor_mul(wp[:, 0:W], z[:, 0:W], mcol)
                    a_col = small.tile([128, 1], F32)
                    nc.vector.tensor_reduce(
                        a_col[:], wp[:, 0:W], axis=mybir.AxisListType.X,
                        op=mybir.AluOpType.add,
                    )
                    # wt = (A^(1/alpha) * cnt^beta)^gamma
                    #    = exp(gamma*(ln(A)/alpha + beta*ln(cnt)))
                    la = small.tile([128, 1], F32)
                    nc.scalar.activation(la[:], a_col[:], AF.Log)
                    lc = small.tile([128, 1], F32)
                    nc.scalar.activation(lc[:], cnt_sb[:, t:t + 1], AF.Log)
                    nc.vector.scalar_tensor_tensor(
                        la[:], lc[:], float(alpha * beta), la[:],
                        op0=mybir.AluOpType.mult, op1=mybir.AluOpType.add,
                    )
                    nc.scalar.activation(
                        wt_col, la[:], AF.Exp, scale=float(gamma / alpha)
                    )

                # X_t = [wt*q | wt]
                xt = xpool.tile([128, 129], BF16)
                nc.vector.tensor_scalar_mul(xt[:, 0:E], qw_sb[:, t, :], wt_col)
                nc.vector.tensor_scalar_add(xt[:, E:E + 1], wt_col, 0.0)

                oh2 = oh_groups[og]
                # leftovers of this tile into previous tile's bank (closes it)
                if t >= 1:
                    nc.tensor.matmul(
                        bank_ps[t - 1][0:SLOTS, :], oh2[:, ogi, 0:SLOTS],
                        xt[:], start=False, stop=True,
                    )
                    nc.scalar.copy(
                        bank_sb[0:SLOTS, t - 1, :], bank_ps[t - 1][0:SLOTS, :]
                    )
                ps = pseg.tile([128, 129], F32)
                bank_ps[t] = ps
                last = t == NT - 1
                nc.tensor.matmul(
                    ps[0:SLOTS, :], oh2[:, ogi, SLOTS:2 * SLOTS], xt[:],
                    start=True, stop=last,
                )
                if last:
                    nc.scalar.copy(bank_sb[0:SLOTS, t, :], ps[0:SLOTS, :])

            # ---- phase C: gather num/den per row, dot with q ----
            def phase_c(t):
                og, ogi = divmod(t, GO)
                if ogi == 0:
                    ogn = min(GO, NT - t)
                    ogt = ohgpool.tile([128, GO, 256], FP8)
                    nc.sync.dma_start(
                        ogt[0:SLOTS, 0:ogn, :], ohs_gat[:, t:t + ogn, :]
                    )
                    oh_groups[("g", og)] = ogt
                g2 = oh_groups[("g", og)]
                gp = pgat.tile([128, 129], F32)
                if t >= 1:
                    nc.tensor.matmul(
                        gp[:], g2[0:SLOTS, ogi, 0:128],
                        bank_sb[0:SLOTS, t - 1, :],
                        start=True, stop=False,
                    )
                    nc.tensor.matmul(
                        gp[:], g2[0:SLOTS, ogi, 128:256],
                        bank_sb[0:SLOTS, t, :],
                        start=False, stop=True,
                    )
                else:
                    nc.tensor.matmul(
                        gp[:], g2[0:SLOTS, ogi, 128:256],
                        bank_sb[0:SLOTS, t, :],
                        start=True, stop=True,
                    )
                nc.scalar.copy(den_sb[:, t:t + 1], gp[:, E:E + 1])
                # rnum = sum_e num[user] * q, fused multiply+accumulate
                pq = zpool.tile([128, E], BF16, tag="pq")
                nc.vector.scalar_tensor_tensor(
                    pq[:], gp[:, 0:E], 0.0, qw_sb[:, t, :],
                    op0=mybir.AluOpType.add, op1=mybir.AluOpType.mult,
                    accum_out=rn_sb[:, t:t + 1],
                )
                if ogi == GO - 1 or t == NT - 1:
                    # finalize this group: r = rnum / den, stream it out
                    t0 = og * GO
                    gn = t - t0 + 1
                    rec = small.tile([128, GO], F32, tag="rec")
                    nc.vector.reciprocal(rec[:, 0:gn], den_sb[:, t0:t0 + gn])
                    nc.vector.tensor_mul(
                        r_sb[:, t0:t0 + gn], rn_sb[:, t0:t0 + gn],
                        rec[:, 0:gn],
                    )
                    # Pool-engine (SWDGE) queue: its dependency wait must not
                    # head-of-line block the R stream DMAs on the SP queue
                    nc.gpsimd.dma_start(
                        r_out[:, t0:t0 + gn], r_sb[:, t0:t0 + gn]
                    )

            # interleave: phase C lags phase A by LAG tiles so its DMAs and
            # matmuls overlap the R stream instead of queueing after it
            LAG = 1
            for it in range(NT + LAG):
                if it < NT:
                    phase_a(it)
                if it >= LAG:
                    phase_c(it - LAG)

    nc.compile()
    return nc


# ----------------------------------------------------------------------------
# entry point
# ----------------------------------------------------------------------------

def kernel(users, items, R_ui, mask, w, item_emb, alpha, beta, gamma,
           _return_extras=False, _trace=False):
    users = np.asarray(users, np.int64)
    items = np.asarray(items, np.int64)
    R_ui = np.asarray(R_ui, np.float32)
    mask_b = np.asarray(mask)
    mask_f = mask_b.astype(np.float32)
    w = np.asarray(w, np.float32)
    item_emb = np.asarray(item_emb, np.float32)
    al = float(np.asarray(alpha).reshape(-1)[0])
    be = float(np.asarray(beta).reshape(-1)[0])
    ga = float(np.asarray(gamma).reshape(-1)[0])

    import time as _time

    t0 = _time.perf_counter()
    in_maps, metas, NT = _preprocess(users, items, R_ui, mask_f, w, item_emb)
    t1 = _time.perf_counter()
    nc = build_program(NT, al, be, ga)
    t2 = _time.perf_counter()
    res = run_bass_kernel_spmd(
        nc, in_maps, core_ids=list(range(N_CORES)), trace=_trace
    )
    t3 = _time.perf_counter()
    print(
        f"[kernel] preprocess {t1-t0:.1f}s  build+schedule {t2-t1:.1f}s  "
        f"compile+run {t3-t2:.1f}s"
    )

    n = users.shape[0]
    r = np.empty(n, np.float32)
    for c in range(N_CORES):
        p, nc_rows = metas[c]
        shard = res.results[c]["r_out"].T.reshape(-1)[:nc_rows]
        r[p] = shard
    if _return_extras:
        return r, res
    return r


# revision 38
# speedup vs baseline: 6.9453x; 1.0399x over previous
"""CDMF segment-reduce kernel for 8 Trainium2 NeuronCores.

Strategy
--------
Host (cheap, index-only + one big gather):
  * stable-sort rows by user id; cut the 100k rows into 8 shards at user
    boundaries ("expert-style sharding of user segments") so each core owns a
    disjoint user range -> no cross-core reduction needed at all.
  * pad every shard to NT*128 rows (mask=0 rows contribute exactly 0).
  * compact each row's valid (mask=1) seq-slices to the front; masked slices
    are never used by the math, so they are not transferred at all. Per-tile
    chunk counts CH[t] (shared across cores) bound the compacted width.
  * pre-gather q = item_emb[items] per shard; R in fp8e4m3, rest bf16.
  * transpose R into PE-friendly chunks RT[(c%2)*64+d, tile, j, row] so the
    feature contraction runs on the tensor engine.
  * build per-tile one-hot matrices (fp8, values 0/1 exact) mapping the 128
    rows of a tile to user-slots of a PSUM "bank" (bank b = users first seen
    in tile b). Only SLOTS (max users/tile, rounded up) slots are kept.

Device (one SPMD program on 8 cores):
  * PE: per tile, CH[t] accumulating matmuls lhsT=RT-chunk [K=128=(2 slices
    x 64 d), M=128 rows], rhs=w-pattern [128, W] -> Z [128 rows, W] in PSUM.
  * DVE: one fused scalar_tensor_tensor (max(Z,tau) * maskc, accum) ->
    per-row weight wt; two tensor_scalar ops build X = [wt*q | wt].
  * PE one-hot matmuls accumulate per-user [sum wt*q | sum wt] (N=129) into
    PSUM banks; ACT flushes each bank to SBUF (bf16).
  * transposed one-hot matmuls gather num[user]/den[user] back per row;
    fused scalar_tensor_tensor computes sum_e num*q; reciprocal+mul -> r.
"""

import numpy as np
import ml_dtypes

import concourse.bass as bass
import concourse.tile as tile
from concourse import bacc, mybir
from concourse.bass_utils import run_bass_kernel_spmd

N_CORES = 8
TAU = 0.01
S = 50          # seq_len
D = 64          # n_features
E = 128         # emb_dim
NJ = S // 2     # max PE k-chunks per tile (2 s-slices of 64 features each)
GO = 8          # one-hot / mask / q tiles per DMA
F32 = mybir.dt.float32
BF16 = mybir.dt.bfloat16
FP8 = mybir.dt.float8e4
# R stream dtype: fp8 e3m4 with an exact 2x pre-scale (2 folded into R,
# 1/2 into w) -- 4 mantissa bits halve the Z error vs e4m3 at equal bytes.
R_DT = mybir.dt.float8e3
R_SCALE = 2.0
R_CLIP = 15.5 / R_SCALE
# q stream: same e3m4 trick with a 32x pre-scale (item_emb ~ N(0, 0.1^2)).
# The two q factors in r = (sum wt*q)(q)/den make r scale by 32^2; scaling
# the den column of X by 32^2 cancels it exactly (powers of two).
Q_DT = mybir.dt.float8e3
NP_Q = mybir.dt.np(Q_DT)
Q_SCALE = 32.0
Q_CLIP = 15.5 / Q_SCALE
DEN_SCALE = Q_SCALE * Q_SCALE

NP_BF16 = ml_dtypes.bfloat16
NP_FP8 = mybir.dt.np(FP8)
NP_R = mybir.dt.np(R_DT)


# R-tile DMA group plan: small groups at both ends to shorten the pipeline
# prologue (first compute waits on a small DMA) and epilogue (last tile's
# chain starts as early as possible).
def _group_plan(NT):
    front, tail = [1, 1, 2], [2, 1]
    if NT <= sum(front) + sum(tail):
        plan, acc = [], 0
        for g in front + tail:
            if acc >= NT:
                break
            plan.append(min(g, NT - acc))
            acc += plan[-1]
        return plan
    mid = NT - sum(front) - sum(tail)
    plan = front + [4] * (mid // 4)
    if mid % 4:
        plan.append(mid % 4)
    return plan + tail


# Data-dependent build parameters (set by _preprocess, read by
# build_program's defaults so `build_program(NT)` builds the same program
# that kernel() runs).
_BUILD_PARAMS = None


# ----------------------------------------------------------------------------
# host-side preprocessing
# ----------------------------------------------------------------------------

def _preprocess(users, items, R_ui, mask, w, item_emb):
    global _BUILD_PARAMS
    n = users.shape[0]
    perm = np.argsort(users, kind="stable")
    users_s = users[perm]

    # shard cuts at user boundaries
    cuts = [0]
    for c in range(1, N_CORES):
        t = round(c * n / N_CORES)
        while 0 < t < n and users_s[t] == users_s[t - 1]:
            t += 1
        cuts.append(min(t, n))
    cuts.append(n)
    sizes = [cuts[c + 1] - cuts[c] for c in range(N_CORES)]
    NT = max(1, int(np.ceil(max(sizes) / 128)))
    NPAD = NT * 128

    q_full = item_emb[items]  # [n, E]
    w_bf = np.asarray(w, NP_BF16)

    # ---- pass 1: per-core masks -> shared chunk counts + slot count ----
    mks = []
    cnts = []
    slots_needed = 1
    metas = []
    row_maps = []
    for c in range(N_CORES):
        lo, hi = cuts[c], cuts[c + 1]
        nc_rows = hi - lo
        p = perm[lo:hi]
        mk = np.zeros((NPAD, S), np.float32)
        mk[:nc_rows] = mask[p]
        mks.append(mk)
        cnts.append(mk.sum(1).astype(np.int64))
        metas.append((p, nc_rows))

        u = np.empty(NPAD, np.int64)
        u[:nc_rows] = users_s[lo:hi]
        u[nc_rows:] = u[nc_rows - 1] if nc_rows > 0 else 0
        uniq, first_idx, inv = np.unique(u, return_index=True,
                                         return_inverse=True)
        ft = first_idx // 128
        gstart = np.searchsorted(ft, ft, side="left")
        slot = np.arange(len(uniq)) - gstart
        slots_needed = max(slots_needed, int(np.bincount(ft).max()))
        row_maps.append((ft[inv], slot[inv]))

    cnt_mat = np.stack(cnts)                        # [cores, NPAD]
    tile_max = cnt_mat.reshape(N_CORES, NT, 128).max(-1).max(0)  # [NT]
    CH = np.maximum(1, (tile_max + 1) // 2).astype(np.int64)     # chunks/tile
    OFF = np.concatenate([[0], np.cumsum(CH)])       # chunk offsets
    TOT = int(OFF[-1])
    Wt = 2 * CH                                      # Z columns per tile
    WOFF = np.concatenate([[0], np.cumsum(Wt)])
    WTOT = int(WOFF[-1])
    SLOTS = int(min(128, ((slots_needed + 31) // 32) * 32))
    assert slots_needed <= 128, "bank overflow"

    _BUILD_PARAMS = {
        "NT": NT,
        "CH": tuple(int(x) for x in CH),
        "SLOTS": SLOTS,
    }

    # ---- pass 2: per-core arrays ----
    in_maps = []
    for c in range(N_CORES):
        p, nc_rows = metas[c]
        mk = mks[c]
        cnt = cnts[c]

        # compact valid slices to the front of each row (stable order)
        Rp = np.zeros((NPAD, S, D), NP_R)
        Rp[:nc_rows] = np.clip(R_ui[p], -R_CLIP, R_CLIP) * R_SCALE
        vidx = np.argsort(mk <= 0, axis=1, kind="stable")  # valid first
        cmp = np.take_along_axis(Rp, vidx[:, :, None], axis=1)
        # zero the dead tail so padded Z columns are exactly 0 -> max()=tau
        cmp[np.arange(S)[None, :] >= cnt[:, None]] = 0

        RT = np.empty((128, TOT * 128), NP_R)
        for t in range(NT):
            ch = int(CH[t])
            blk = cmp[t * 128:(t + 1) * 128, 0:2 * ch, :]
            RT[:, OFF[t] * 128:OFF[t + 1] * 128] = (
                blk.reshape(128, ch, 2, D).transpose(2, 3, 1, 0)
                .reshape(128, ch * 128)
            )

        # the compacted mask (column c of tile t live iff c < cnt[row]) is
        # built on-device from cnt via an iota compare -- only cnt ships.
        cntw = np.ascontiguousarray(
            cnt.reshape(NT, 128).T, np.float32)      # [128, NT]
        iota50 = np.ascontiguousarray(
            np.broadcast_to(np.arange(S, dtype=np.float32), (128, S))
        ).astype(NP_BF16)
        # fast path: padded Z columns contribute exactly tau each, so
        # wt = cnt*(acc - (W-cnt)*tau); ship cnt and the correction,
        # both pre-scaled by DEN_SCALE for the X den column.
        cm = cntw.astype(np.float32)
        Wrow = (2 * CH).astype(np.float32)[None, :]
        cnt2 = np.ascontiguousarray(cm * DEN_SCALE)
        corr = np.ascontiguousarray(
            np.float32(TAU) * cm * (cm - Wrow) * DEN_SCALE, np.float32)

        qp = np.zeros((NPAD, E), NP_Q)
        qp[:nc_rows] = np.clip(q_full[p], -Q_CLIP, Q_CLIP) * Q_SCALE
        qw = np.ascontiguousarray(qp.reshape(NT, 128, E).transpose(1, 0, 2))

        # w-pattern for the PE feature contraction: [128=(c%2)*64+d, NJ, S]
        # (carries the 1/R_SCALE compensation -- exact, power of two)
        wpat = np.zeros((128, NJ, S), NP_BF16)
        wb = (w_bf.astype(np.float32) / R_SCALE).astype(NP_BF16)
        for sp in range(2):
            for j in range(NJ):
                wpat[sp * 64:(sp + 1) * 64, j, 2 * j + sp] = wb

        fr, sr = row_maps[c]
        ii = np.arange(NPAD)
        tt, kk = ii // 128, ii % 128
        own = fr == tt
        prev = fr == tt - 1
        assert np.all(own | prev), "user spans >2 tiles (unexpected padding)"

        seg = np.zeros((128, NT, 2 * SLOTS), NP_FP8)
        gat = np.zeros((SLOTS, NT, 256), NP_FP8)
        seg[kk[own], tt[own], SLOTS + sr[own]] = 1.0
        seg[kk[prev], tt[prev], sr[prev]] = 1.0
        gat[sr[own], tt[own], 128 + kk[own]] = 1.0
        gat[sr[prev], tt[prev], kk[prev]] = 1.0

        in_maps.append(
            {
                "RT": RT,
                "cntw": cntw,
                "cnt2": cnt2,
                "corr": corr,
                "iota50": iota50,
                "qw": qw,
                "wpat": wpat,
                "ohs_seg": seg,
                "ohs_gat": gat,
            }
        )
    return in_maps, metas, NT


# ----------------------------------------------------------------------------
# device program
# ----------------------------------------------------------------------------

def build_program(NT, alpha=1.0, beta=1.0, gamma=1.0, params=None):
    if params is None:
        params = _BUILD_PARAMS
    if params is None or params["NT"] != NT:
        params = {"NT": NT, "CH": (NJ,) * NT, "SLOTS": 128}
    CH = params["CH"]
    SLOTS = params["SLOTS"]
    OFF = [0]
    for ch in CH:
        OFF.append(OFF[-1] + ch)
    TOT = OFF[-1]
    WOFF = [2 * o for o in OFF]
    WTOT = 2 * TOT

    nc = bacc.Bacc(
        "TRN2", target_bir_lowering=False, debug=False, num_devices=N_CORES
    )

    RT = nc.dram_tensor("RT", [128, TOT * 128], R_DT, kind="ExternalInput")
    cntw = nc.dram_tensor("cntw", [128, NT], F32, kind="ExternalInput")
    cnt2 = nc.dram_tensor("cnt2", [128, NT], F32, kind="ExternalInput")
    corr = nc.dram_tensor("corr", [128, NT], F32, kind="ExternalInput")
    iota50 = nc.dram_tensor("iota50", [128, S], BF16, kind="ExternalInput")
    qw = nc.dram_tensor("qw", [128, NT, E], Q_DT, kind="ExternalInput")
    wpat = nc.dram_tensor("wpat", [128, NJ, S], BF16, kind="ExternalInput")
    ohs_seg = nc.dram_tensor("ohs_seg", [128, NT, 2 * SLOTS], FP8,
                             kind="ExternalInput")
    ohs_gat = nc.dram_tensor("ohs_gat", [SLOTS, NT, 256], FP8,
                             kind="ExternalInput")
    r_out = nc.dram_tensor("r_out", [128, NT], F32, kind="ExternalOutput")

    fast = (alpha == 1.0) and (beta == 1.0) and (gamma == 1.0)
    AF = mybir.ActivationFunctionType

    plan = _group_plan(NT)
    gstart = [0]
    for g in plan:
        gstart.append(gstart[-1] + g)
    MAXC = max(OFF[gstart[i + 1]] - OFF[gstart[i]] for i in range(len(plan)))

    with tile.TileContext(nc) as tc:
        with (
            tc.tile_pool(name="const", bufs=1) as constp,
            tc.tile_pool(name="rpool", bufs=4) as rpool,
            tc.tile_pool(name="zpool", bufs=2) as zpool,
            tc.tile_pool(name="mpool", bufs=2) as mpool,
            tc.tile_pool(name="small", bufs=8) as small,
            tc.tile_pool(name="xpool", bufs=4) as xpool,
            tc.tile_pool(name="ohpool", bufs=3) as ohpool,
            tc.tile_pool(name="ohgpool", bufs=4) as ohgpool,
            tc.tile_pool(name="banks", bufs=1) as bankp,
            tc.tile_pool(name="psum_z", bufs=2, space="PSUM") as pz,
            tc.tile_pool(name="psum_seg", bufs=3, space="PSUM") as pseg,
            tc.tile_pool(name="psum_gat", bufs=3, space="PSUM") as pgat,
        ):
            wpat_sb = constp.tile([128, NJ, S], BF16)
            nc.sync.dma_start(wpat_sb[:], wpat[:, :, :])
            iota_sb = constp.tile([128, S], BF16)
            nc.sync.dma_start(iota_sb[:], iota50[:, :])
            qw_sb = constp.tile([128, NT, E], Q_DT)
            cnt_sb = constp.tile([128, NT], F32)
            nc.sync.dma_start(cnt_sb[:], cntw[:, :])
            cnt2_sb = constp.tile([128, NT], F32)
            nc.sync.dma_start(cnt2_sb[:], cnt2[:, :])
            corr_sb = constp.tile([128, NT], F32)
            nc.sync.dma_start(corr_sb[:], corr[:, :])
            den_sb = constp.tile([128, NT], F32)
            wt_sb = constp.tile([128, NT], F32)
            rn_sb = constp.tile([128, NT], F32)
            r_sb = constp.tile([128, NT], F32)
            bank_sb = bankp.tile([128, NT, 129], BF16)

            r_groups = {}
            oh_groups = {}
            bank_ps = [None] * NT
            tile_group = []
            for gi_, g in enumerate(plan):
                tile_group += [gi_] * g

            def phase_a(t):
                g = tile_group[t]
                if t == gstart[g]:
                    t1 = gstart[g + 1]
                    c0, c1 = OFF[t], OFF[t1]
                    rg = rpool.tile([128, MAXC * 128], R_DT)
                    nc.sync.dma_start(
                        rg[:, 0:(c1 - c0) * 128], RT[:, c0 * 128:c1 * 128]
                    )
                    r_groups[g] = rg
                og, ogi = divmod(t, GO)
                if ogi == 0:
                    ogn = min(GO, NT - t)
                    osg = ohpool.tile([128, GO, 2 * SLOTS], FP8)
                    nc.sync.dma_start(
                        osg[:, 0:ogn, :], ohs_seg[:, t:t + ogn, :]
                    )
                    oh_groups[og] = osg
                    nc.sync.dma_start(
                        qw_sb[:, t:t + ogn, :], qw[:, t:t + ogn, :]
                    )

                W = 2 * CH[t]
                rg = r_groups[g]
                base = (OFF[t] - OFF[gstart[g]]) * 128
                zps = pz.tile([128, S], F32)
                for j in range(CH[t]):
                    nc.tensor.matmul(
                        zps[:, 0:W], rg[:, base + j * 128:base + (j + 1) * 128],
                        wpat_sb[:, j, 0:W],
                        start=(j == 0), stop=(j == CH[t] - 1),
                    )

                wt_col = wt_sb[:, t:t + 1]
                cnt_col = cnt_sb[:, t:t + 1]
                if fast:
                    # acc = sum_c max(z, tau); padded columns add exactly tau
                    # so den = DEN_SCALE*wt = acc*cnt2 + corr (see host)
                    acc_col = small.tile([128, 1], F32, tag="acc")
                    wp = zpool.tile([128, S], BF16)
                    nc.vector.tensor_scalar(
                        wp[:, 0:W], zps[:, 0:W], TAU, None,
                        op0=mybir.AluOpType.max, op1=mybir.AluOpType.add,
                        accum_out=acc_col[:],
                    )
                else:
                    mct = mpool.tile([128, S], BF16)
                    mcol = mct[:, 0:W]
                    nc.vector.tensor_scalar(
                        mcol, iota_sb[:, 0:W], cnt_col, 1.0,
                        op0=mybir.AluOpType.is_lt, op1=mybir.AluOpType.mult,
                    )
                    z = zpool.tile([128, S], F32, tag="zf32")
                    nc.vector.tensor_scalar_max(z[:, 0:W], zps[:, 0:W], TAU)
                    # z <- exp(alpha * ln z)   (z >= TAU > 0)
                    nc.scalar.activation(z[:, 0:W], z[:, 0:W], AF.Log)
                    nc.scalar.activation(z[:, 0:W], z[:, 0:W], AF.Exp,
                                         scale=float(alpha))
                    wp = zpool.tile([128, S], F32, tag="wpf32")
                    nc.vector.tensor_mul(wp[:, 0:W], z[:, 0:W], mcol)
                    a_col = small.tile([128, 1], F32)
                    nc.vector.tensor_reduce(
                        a_col[:], wp[:, 0:W], axis=mybir.AxisListType.X,
                        op=mybir.AluOpType.add,
                    )
                    # wt = (A^(1/alpha) * cnt^beta)^gamma
                    #    = exp(gamma*(ln(A)/alpha + beta*ln(cnt)))
                    la = small.tile([128, 1], F32)
                    nc.scalar.activation(la[:], a_col[:], AF.Log)
                    lc = small.tile([128, 1], F32)
                    nc.scalar.activation(lc[:], cnt_sb[:, t:t + 1], AF.Log)
                    nc.vector.scalar_tensor_tensor(
                        la[:], lc[:], float(alpha * beta), la[:],
                        op0=mybir.AluOpType.mult, op1=mybir.AluOpType.add,
                    )
                    nc.scalar.activation(
                        wt_col, la[:], AF.Exp, scale=float(gamma / alpha)
                    )

                # X_t = [wt*q | DEN_SCALE*wt]; dwt_col = DEN_SCALE*wt (f32)
                xt = xpool.tile([128, 129], BF16)
                dwt_col = wt_sb[:, t:t + 1]
                if fast:
                    nc.vector.scalar_tensor_tensor(
                        dwt_col, acc_col[:], cnt2_sb[:, t:t + 1],
                        corr_sb[:, t:t + 1],
                        op0=mybir.AluOpType.mult, op1=mybir.AluOpType.add,
                    )
                else:
                    nc.vector.tensor_scalar_mul(dwt_col, wt_col, DEN_SCALE)
                nc.vector.tensor_scalar_add(xt[:, E:E + 1], dwt_col, 0.0)
                # q column: q * wt = q * denwt / DEN_SCALE
                nc.vector.tensor_scalar(
                    xt[:, 0:E], qw_sb[:, t, :], dwt_col,
                    1.0 / DEN_SCALE, op0=mybir.AluOpType.mult,
                    op1=mybir.AluOpType.mult,
                )

                oh2 = oh_groups[og]
                # leftovers of this tile into previous tile's bank (closes it)
                if t >= 1:
                    nc.tensor.matmul(
                        bank_ps[t - 1][0:SLOTS, :], oh2[:, ogi, 0:SLOTS],
                        xt[:], start=False, stop=True,
                    )
                    nc.scalar.copy(
                        bank_sb[0:SLOTS, t - 1, :], bank_ps[t - 1][0:SLOTS, :]
                    )
                ps = pseg.tile([128, 129], F32)
                bank_ps[t] = ps
                last = t == NT - 1
                nc.tensor.matmul(
                    ps[0:SLOTS, :], oh2[:, ogi, SLOTS:2 * SLOTS], xt[:],
                    start=True, stop=last,
                )
                if last:
                    nc.scalar.copy(bank_sb[0:SLOTS, t, :], ps[0:SLOTS, :])

            # ---- phase C: gather num/den per row, dot with q ----
            NOG = (NT + GO - 1) // GO

            def ensure_ohg(og):
                if og >= NOG or ("g", og) in oh_groups:
                    return
                t0 = og * GO
                ogn = min(GO, NT - t0)
                ogt = ohgpool.tile([128, GO, 256], FP8)
                nc.sync.dma_start(
                    ogt[0:SLOTS, 0:ogn, :], ohs_gat[:, t0:t0 + ogn, :]
                )
                oh_groups[("g", og)] = ogt

            def phase_c(t):
                og, ogi = divmod(t, GO)
                if ogi == 0:
                    ensure_ohg(og)
                g2 = oh_groups[("g", og)]
                gp = pgat.tile([128, 129], F32)
                if t >= 1:
                    nc.tensor.matmul(
                        gp[:], g2[0:SLOTS, ogi, 0:128],
                        bank_sb[0:SLOTS, t - 1, :],
                        start=True, stop=False,
                    )
                    nc.tensor.matmul(
                        gp[:], g2[0:SLOTS, ogi, 128:256],
                        bank_sb[0:SLOTS, t, :],
                        start=False, stop=True,
                    )
                else:
                    nc.tensor.matmul(
                        gp[:], g2[0:SLOTS, ogi, 128:256],
                        bank_sb[0:SLOTS, t, :],
                        start=True, stop=True,
                    )
                nc.scalar.copy(den_sb[:, t:t + 1], gp[:, E:E + 1])
                # rnum = sum_e num[user] * q, fused multiply+accumulate
                pq = zpool.tile([128, E], BF16, tag="pq")
                nc.vector.scalar_tensor_tensor(
                    pq[:], gp[:, 0:E], 0.0, qw_sb[:, t, :],
                    op0=mybir.AluOpType.add, op1=mybir.AluOpType.mult,
                    accum_out=rn_sb[:, t:t + 1],
                )
                if ogi == GO - 1 or t == NT - 1:
                    # finalize this group: r = rnum / den, stream it out
                    t0 = og * GO
                    gn = t - t0 + 1
                    rec = small.tile([128, GO], F32, tag="rec")
                    nc.vector.reciprocal(rec[:, 0:gn], den_sb[:, t0:t0 + gn])
                    nc.vector.tensor_mul(
                        r_sb[:, t0:t0 + gn], rn_sb[:, t0:t0 + gn],
                        rec[:, 0:gn],
                    )
                    # Pool-engine (SWDGE) queue: its dependency wait must not
                    # head-of-line block the R stream DMAs on the SP queue.
                    # The final group rides SP (cheaper, queue is empty then).
                    eng = nc.sync if t == NT - 1 else nc.gpsimd
                    eng.dma_start(
                        r_out[:, t0:t0 + gn], r_sb[:, t0:t0 + gn]
                    )

            # interleave: phase C lags phase A by LAG tiles so its DMAs and
            # matmuls overlap the R stream instead of queueing after it
            LAG = 1
            for it in range(NT + LAG):
                if it < NT:
                    phase_a(it)
                if it >= LAG:
                    phase_c(it - LAG)

    nc.compile()
    return nc


# ----------------------------------------------------------------------------
# entry point
# ----------------------------------------------------------------------------

def kernel(users, items, R_ui, mask, w, item_emb, alpha, beta, gamma,
           _return_extras=False, _trace=False):
    users = np.asarray(users, np.int64)
    items = np.asarray(items, np.int64)
    R_ui = np.asarray(R_ui, np.float32)
    mask_b = np.asarray(mask)
    mask_f = mask_b.astype(np.float32)
    w = np.asarray(w, np.float32)
    item_emb = np.asarray(item_emb, np.float32)
    al = float(np.asarray(alpha).reshape(-1)[0])
    be = float(np.asarray(beta).reshape(-1)[0])
    ga = float(np.asarray(gamma).reshape(-1)[0])

    import time as _time

    t0 = _time.perf_counter()
    in_maps, metas, NT = _preprocess(users, items, R_ui, mask_f, w, item_emb)
    t1 = _time.perf_counter()
    nc = build_program(NT, al, be, ga)
    t2 = _time.perf_counter()
    res = run_bass_kernel_spmd(
        nc, in_maps, core_ids=list(range(N_CORES)), trace=_trace
    )
    t3 = _time.perf_counter()
    print(
        f"[kernel] preprocess {t1-t0:.1f}s  build+schedule {t2-t1:.1f}s  "
        f"compile+run {t3-t2:.1f}s"
    )

    n = users.shape[0]
    r = np.empty(n, np.float32)
    for c in range(N_CORES):
        p, nc_rows = metas[c]
        shard = res.results[c]["r_out"].T.reshape(-1)[:nc_rows]
        r[p] = shard
    if _return_extras:
        return r, res
    return r


# revision 44
# speedup vs baseline: 7.0251x; 1.0115x over previous
"""CDMF segment-reduce kernel for 8 Trainium2 NeuronCores.

Strategy
--------
Host (cheap, index-only + one big gather):
  * stable-sort rows by user id; cut the 100k rows into 8 shards at user
    boundaries ("expert-style sharding of user segments") so each core owns a
    disjoint user range -> no cross-core reduction needed at all.
  * pad every shard to NT*128 rows (mask=0 rows contribute exactly 0).
  * compact each row's valid (mask=1) seq-slices to the front and zero the
    dead tail; masked slices are never used by the math, so they are not
    transferred at all. Per-tile chunk counts CH[t] (shared across all
    cores, the program is SPMD) bound the compacted width.
  * pre-gather q = item_emb[items] per shard. R and q ship as fp8 e3m4
    with exact power-of-2 pre-scales (compensated in w / the den column),
    one-hots as fp8 0/1, the rest bf16.
  * transpose R into PE-friendly chunks RT[(c%2)*64+d, tile, j, row] so the
    feature contraction runs on the tensor engine.
  * build per-tile one-hot matrices mapping the 128 rows of a tile to
    user-slots of a PSUM "bank" (bank b = users first seen in tile b).
    Only SLOTS (max users/tile, rounded up to 32) slots are kept.

Device (one SPMD program on 8 cores, DMA-roofline bound):
  * PE: per tile, CH[t] accumulating matmuls lhsT=RT-chunk [K=128=(2 slices
    x 64 d), M=128 rows], rhs=w-pattern [128, W] -> Z [128 rows, W] in PSUM.
  * DVE: one tensor_scalar (max(Z,tau), sum-accum) -> acc; padded columns
    contribute exactly tau each, so den = DEN_SCALE*wt = acc*cnt2 + corr
    with host-precomputed per-row constants (no mask tensor at all).
  * PE one-hot matmuls accumulate per-user [sum wt*q | DEN_SCALE*sum wt]
    (N=129) into PSUM banks; ACT flushes each bank to SBUF (bf16).
  * transposed one-hot matmuls gather num[user]/den[user] back per row;
    fused scalar_tensor_tensor computes sum_e num*q; per-GO-group
    reciprocal+mul finalizes r and streams it out on the idle Pool queue.
  * phase C lags phase A by LAG tiles inside one program-order loop so its
    DMAs interleave with the R stream instead of queueing after it.
"""

import numpy as np
import ml_dtypes

import concourse.bass as bass
import concourse.tile as tile
from concourse import bacc, mybir
from concourse.bass_utils import run_bass_kernel_spmd

N_CORES = 8
TAU = 0.01
S = 50          # seq_len
D = 64          # n_features
E = 128         # emb_dim
NJ = S // 2     # max PE k-chunks per tile (2 s-slices of 64 features each)
GO = 16          # one-hot / mask / q tiles per DMA
F32 = mybir.dt.float32
BF16 = mybir.dt.bfloat16
FP8 = mybir.dt.float8e4
# R stream dtype: fp8 e3m4 with an exact 2x pre-scale (2 folded into R,
# 1/2 into w) -- 4 mantissa bits halve the Z error vs e4m3 at equal bytes.
R_DT = mybir.dt.float8e3
R_SCALE = 2.0
R_CLIP = 15.5 / R_SCALE
# q stream: same e3m4 trick with a 32x pre-scale (item_emb ~ N(0, 0.1^2)).
# The two q factors in r = (sum wt*q)(q)/den make r scale by 32^2; scaling
# the den column of X by 32^2 cancels it exactly (powers of two).
Q_DT = mybir.dt.float8e3
NP_Q = mybir.dt.np(Q_DT)
Q_SCALE = 32.0
Q_CLIP = 15.5 / Q_SCALE
DEN_SCALE = Q_SCALE * Q_SCALE

NP_BF16 = ml_dtypes.bfloat16
NP_FP8 = mybir.dt.np(FP8)
NP_R = mybir.dt.np(R_DT)


# R-tile DMA group plan: small groups at both ends to shorten the pipeline
# prologue (first compute waits on a small DMA) and epilogue (last tile's
# chain starts as early as possible).
def _group_plan(NT):
    front, tail = [1, 1, 2], [2, 2, 1, 1]
    if NT <= sum(front) + sum(tail):
        plan, acc = [], 0
        for g in front + tail:
            if acc >= NT:
                break
            plan.append(min(g, NT - acc))
            acc += plan[-1]
        return plan
    mid = NT - sum(front) - sum(tail)
    plan = front + [4] * (mid // 4)
    if mid % 4:
        plan.append(mid % 4)
    return plan + tail


# Data-dependent build parameters (set by _preprocess, read by
# build_program's defaults so `build_program(NT)` builds the same program
# that kernel() runs).
_BUILD_PARAMS = None


# ----------------------------------------------------------------------------
# host-side preprocessing
# ----------------------------------------------------------------------------

def _preprocess(users, items, R_ui, mask, w, item_emb):
    global _BUILD_PARAMS
    n = users.shape[0]
    perm = np.argsort(users, kind="stable")
    users_s = users[perm]

    # shard cuts at user boundaries
    cuts = [0]
    for c in range(1, N_CORES):
        t = round(c * n / N_CORES)
        while 0 < t < n and users_s[t] == users_s[t - 1]:
            t += 1
        cuts.append(min(t, n))
    cuts.append(n)
    sizes = [cuts[c + 1] - cuts[c] for c in range(N_CORES)]
    NT = max(1, int(np.ceil(max(sizes) / 128)))
    NPAD = NT * 128

    q_full = item_emb[items]  # [n, E]
    w_bf = np.asarray(w, NP_BF16)

    # ---- pass 1: per-core masks -> shared chunk counts + slot count ----
    mks = []
    cnts = []
    slots_needed = 1
    metas = []
    row_maps = []
    for c in range(N_CORES):
        lo, hi = cuts[c], cuts[c + 1]
        nc_rows = hi - lo
        p = perm[lo:hi]
        mk = np.zeros((NPAD, S), np.float32)
        mk[:nc_rows] = mask[p]
        mks.append(mk)
        cnts.append(mk.sum(1).astype(np.int64))
        metas.append((p, nc_rows))

        u = np.empty(NPAD, np.int64)
        u[:nc_rows] = users_s[lo:hi]
        u[nc_rows:] = u[nc_rows - 1] if nc_rows > 0 else 0
        uniq, first_idx, inv = np.unique(u, return_index=True,
                                         return_inverse=True)
        ft = first_idx // 128
        gstart = np.searchsorted(ft, ft, side="left")
        slot = np.arange(len(uniq)) - gstart
        slots_needed = max(slots_needed, int(np.bincount(ft).max()))
        row_maps.append((ft[inv], slot[inv]))

    cnt_mat = np.stack(cnts)                        # [cores, NPAD]
    tile_max = cnt_mat.reshape(N_CORES, NT, 128).max(-1).max(0)  # [NT]
    CH = np.maximum(1, (tile_max + 1) // 2).astype(np.int64)     # chunks/tile
    OFF = np.concatenate([[0], np.cumsum(CH)])       # chunk offsets
    TOT = int(OFF[-1])
    Wt = 2 * CH                                      # Z columns per tile
    WOFF = np.concatenate([[0], np.cumsum(Wt)])
    WTOT = int(WOFF[-1])
    SLOTS = int(min(128, ((slots_needed + 31) // 32) * 32))
    assert slots_needed <= 128, "bank overflow"

    _BUILD_PARAMS = {
        "NT": NT,
        "CH": tuple(int(x) for x in CH),
        "SLOTS": SLOTS,
    }

    # ---- pass 2: per-core arrays ----
    in_maps = []
    for c in range(N_CORES):
        p, nc_rows = metas[c]
        mk = mks[c]
        cnt = cnts[c]

        # compact valid slices to the front of each row (stable order)
        Rp = np.zeros((NPAD, S, D), NP_R)
        Rp[:nc_rows] = np.clip(R_ui[p], -R_CLIP, R_CLIP) * R_SCALE
        vidx = np.argsort(mk <= 0, axis=1, kind="stable")  # valid first
        cmp = np.take_along_axis(Rp, vidx[:, :, None], axis=1)
        # zero the dead tail so padded Z columns are exactly 0 -> max()=tau
        cmp[np.arange(S)[None, :] >= cnt[:, None]] = 0

        RT = np.empty((128, TOT * 128), NP_R)
        for t in range(NT):
            ch = int(CH[t])
            blk = cmp[t * 128:(t + 1) * 128, 0:2 * ch, :]
            RT[:, OFF[t] * 128:OFF[t + 1] * 128] = (
                blk.reshape(128, ch, 2, D).transpose(2, 3, 1, 0)
                .reshape(128, ch * 128)
            )

        # the compacted mask (column c of tile t live iff c < cnt[row]) is
        # built on-device from cnt via an iota compare -- only cnt ships.
        cntw = np.ascontiguousarray(
            cnt.reshape(NT, 128).T, np.float32)      # [128, NT]
        iota50 = np.ascontiguousarray(
            np.broadcast_to(np.arange(S, dtype=np.float32), (128, S))
        ).astype(NP_BF16)
        # fast path: padded Z columns contribute exactly tau each, so
        # wt = cnt*(acc - (W-cnt)*tau); ship cnt and the correction,
        # both pre-scaled by DEN_SCALE for the X den column.
        cm = cntw.astype(np.float32)
        Wrow = (2 * CH).astype(np.float32)[None, :]
        cnt2 = np.ascontiguousarray(cm * DEN_SCALE)
        corr = np.ascontiguousarray(
            np.float32(TAU) * cm * (cm - Wrow) * DEN_SCALE, np.float32)

        qp = np.zeros((NPAD, E), NP_Q)
        qp[:nc_rows] = np.clip(q_full[p], -Q_CLIP, Q_CLIP) * Q_SCALE
        qw = np.ascontiguousarray(qp.reshape(NT, 128, E).transpose(1, 0, 2))

        # w-pattern for the PE feature contraction: [128=(c%2)*64+d, NJ, S]
        # (carries the 1/R_SCALE compensation -- exact, power of two)
        wpat = np.zeros((128, NJ, S), NP_BF16)
        wb = (w_bf.astype(np.float32) / R_SCALE).astype(NP_BF16)
        for sp in range(2):
            for j in range(NJ):
                wpat[sp * 64:(sp + 1) * 64, j, 2 * j + sp] = wb

        fr, sr = row_maps[c]
        ii = np.arange(NPAD)
        tt, kk = ii // 128, ii % 128
        own = fr == tt
        prev = fr == tt - 1
        assert np.all(own | prev), "user spans >2 tiles (unexpected padding)"

        seg = np.zeros((128, NT, 2 * SLOTS), NP_FP8)
        gat = np.zeros((SLOTS, NT, 256), NP_FP8)
        seg[kk[own], tt[own], SLOTS + sr[own]] = 1.0
        seg[kk[prev], tt[prev], sr[prev]] = 1.0
        gat[sr[own], tt[own], 128 + kk[own]] = 1.0
        gat[sr[prev], tt[prev], kk[prev]] = 1.0

        in_maps.append(
            {
                "RT": RT,
                "cntw": cntw,
                "cnt2": cnt2,
                "corr": corr,
                "iota50": iota50,
                "qw": qw,
                "wpat": wpat,
                "ohs_seg": seg,
                "ohs_gat": gat,
            }
        )
    return in_maps, metas, NT


# ----------------------------------------------------------------------------
# device program
# ----------------------------------------------------------------------------

def build_program(NT, alpha=1.0, beta=1.0, gamma=1.0, params=None):
    if params is None:
        params = _BUILD_PARAMS
    if params is None or params["NT"] != NT:
        params = {"NT": NT, "CH": (NJ,) * NT, "SLOTS": 128}
    CH = params["CH"]
    SLOTS = params["SLOTS"]
    OFF = [0]
    for ch in CH:
        OFF.append(OFF[-1] + ch)
    TOT = OFF[-1]
    WOFF = [2 * o for o in OFF]
    WTOT = 2 * TOT

    nc = bacc.Bacc(
        "TRN2", target_bir_lowering=False, debug=False, num_devices=N_CORES
    )

    RT = nc.dram_tensor("RT", [128, TOT * 128], R_DT, kind="ExternalInput")
    cntw = nc.dram_tensor("cntw", [128, NT], F32, kind="ExternalInput")
    cnt2 = nc.dram_tensor("cnt2", [128, NT], F32, kind="ExternalInput")
    corr = nc.dram_tensor("corr", [128, NT], F32, kind="ExternalInput")
    iota50 = nc.dram_tensor("iota50", [128, S], BF16, kind="ExternalInput")
    qw = nc.dram_tensor("qw", [128, NT, E], Q_DT, kind="ExternalInput")
    wpat = nc.dram_tensor("wpat", [128, NJ, S], BF16, kind="ExternalInput")
    ohs_seg = nc.dram_tensor("ohs_seg", [128, NT, 2 * SLOTS], FP8,
                             kind="ExternalInput")
    ohs_gat = nc.dram_tensor("ohs_gat", [SLOTS, NT, 256], FP8,
                             kind="ExternalInput")
    r_out = nc.dram_tensor("r_out", [128, NT], F32, kind="ExternalOutput")

    fast = (alpha == 1.0) and (beta == 1.0) and (gamma == 1.0)
    AF = mybir.ActivationFunctionType

    plan = _group_plan(NT)
    gstart = [0]
    for g in plan:
        gstart.append(gstart[-1] + g)
    MAXC = max(OFF[gstart[i + 1]] - OFF[gstart[i]] for i in range(len(plan)))

    with tile.TileContext(nc) as tc:
        with (
            tc.tile_pool(name="const", bufs=1) as constp,
            tc.tile_pool(name="rpool", bufs=3) as rpool,
            tc.tile_pool(name="zpool", bufs=2) as zpool,
            tc.tile_pool(name="mpool", bufs=2) as mpool,
            tc.tile_pool(name="small", bufs=8) as small,
            tc.tile_pool(name="xpool", bufs=4) as xpool,
            tc.tile_pool(name="ohpool", bufs=3) as ohpool,
            tc.tile_pool(name="ohgpool", bufs=4) as ohgpool,
            tc.tile_pool(name="banks", bufs=1) as bankp,
            tc.tile_pool(name="psum_z", bufs=2, space="PSUM") as pz,
            tc.tile_pool(name="psum_seg", bufs=3, space="PSUM") as pseg,
            tc.tile_pool(name="psum_gat", bufs=3, space="PSUM") as pgat,
        ):
            wpat_sb = constp.tile([128, NJ, S], BF16)
            nc.sync.dma_start(wpat_sb[:], wpat[:, :, :])
            iota_sb = constp.tile([128, S], BF16)
            nc.sync.dma_start(iota_sb[:], iota50[:, :])
            qw_sb = constp.tile([128, NT, E], Q_DT)
            cnt_sb = constp.tile([128, NT], F32)
            nc.sync.dma_start(cnt_sb[:], cntw[:, :])
            cnt2_sb = constp.tile([128, NT], F32)
            nc.sync.dma_start(cnt2_sb[:], cnt2[:, :])
            corr_sb = constp.tile([128, NT], F32)
            nc.sync.dma_start(corr_sb[:], corr[:, :])
            den_sb = constp.tile([128, NT], F32)
            wt_sb = constp.tile([128, NT], F32)
            rn_sb = constp.tile([128, NT], F32)
            r_sb = constp.tile([128, NT], F32)
            bank_sb = bankp.tile([128, NT, 129], BF16)

            r_groups = {}
            oh_groups = {}
            bank_ps = [None] * NT
            tile_group = []
            for gi_, g in enumerate(plan):
                tile_group += [gi_] * g

            def phase_a(t):
                g = tile_group[t]
                if t == gstart[g]:
                    t1 = gstart[g + 1]
                    c0, c1 = OFF[t], OFF[t1]
                    rg = rpool.tile([128, MAXC * 128], R_DT)
                    nc.sync.dma_start(
                        rg[:, 0:(c1 - c0) * 128], RT[:, c0 * 128:c1 * 128]
                    )
                    r_groups[g] = rg
                og, ogi = divmod(t, GO)
                if ogi == 0:
                    ogn = min(GO, NT - t)
                    osg = ohpool.tile([128, GO, 2 * SLOTS], FP8)
                    nc.sync.dma_start(
                        osg[:, 0:ogn, :], ohs_seg[:, t:t + ogn, :]
                    )
                    oh_groups[og] = osg
                    nc.sync.dma_start(
                        qw_sb[:, t:t + ogn, :], qw[:, t:t + ogn, :]
                    )

                W = 2 * CH[t]
                rg = r_groups[g]
                base = (OFF[t] - OFF[gstart[g]]) * 128
                zps = pz.tile([128, S], F32)
                for j in range(CH[t]):
                    nc.tensor.matmul(
                        zps[:, 0:W], rg[:, base + j * 128:base + (j + 1) * 128],
                        wpat_sb[:, j, 0:W],
                        start=(j == 0), stop=(j == CH[t] - 1),
                    )

                wt_col = wt_sb[:, t:t + 1]
                cnt_col = cnt_sb[:, t:t + 1]
                if fast:
                    # acc = sum_c max(z, tau); padded columns add exactly tau
                    # so den = DEN_SCALE*wt = acc*cnt2 + corr (see host)
                    acc_col = small.tile([128, 1], F32, tag="acc")
                    wp = zpool.tile([128, S], BF16)
                    nc.vector.tensor_scalar(
                        wp[:, 0:W], zps[:, 0:W], TAU, None,
                        op0=mybir.AluOpType.max, op1=mybir.AluOpType.add,
                        accum_out=acc_col[:],
                    )
                else:
                    mct = mpool.tile([128, S], BF16)
                    mcol = mct[:, 0:W]
                    nc.vector.tensor_scalar(
                        mcol, iota_sb[:, 0:W], cnt_col, 1.0,
                        op0=mybir.AluOpType.is_lt, op1=mybir.AluOpType.mult,
                    )
                    z = zpool.tile([128, S], F32, tag="zf32")
                    nc.vector.tensor_scalar_max(z[:, 0:W], zps[:, 0:W], TAU)
                    # z <- exp(alpha * ln z)   (z >= TAU > 0)
                    nc.scalar.activation(z[:, 0:W], z[:, 0:W], AF.Log)
                    nc.scalar.activation(z[:, 0:W], z[:, 0:W], AF.Exp,
                                         scale=float(alpha))
                    wp = zpool.tile([128, S], F32, tag="wpf32")
                    nc.vector.tensor_mul(wp[:, 0:W], z[:, 0:W], mcol)
                    a_col = small.tile([128, 1], F32)
                    nc.vector.tensor_reduce(
                        a_col[:], wp[:, 0:W], axis=mybir.AxisListType.X,
                        op=mybir.AluOpType.add,
                    )
                    # wt = (A^(1/alpha) * cnt^beta)^gamma
                    #    = exp(gamma*(ln(A)/alpha + beta*ln(cnt)))
                    la = small.tile([128, 1], F32)
                    nc.scalar.activation(la[:], a_col[:], AF.Log)
                    lc = small.tile([128, 1], F32)
                    nc.scalar.activation(lc[:], cnt_sb[:, t:t + 1], AF.Log)
                    nc.vector.scalar_tensor_tensor(
                        la[:], lc[:], float(alpha * beta), la[:],
                        op0=mybir.AluOpType.mult, op1=mybir.AluOpType.add,
                    )
                    nc.scalar.activation(
                        wt_col, la[:], AF.Exp, scale=float(gamma / alpha)
                    )

                # X_t = [wt*q | DEN_SCALE*wt]; dwt_col = DEN_SCALE*wt (f32)
                xt = xpool.tile([128, 129], BF16)
                dwt_col = wt_sb[:, t:t + 1]
                if fast:
                    nc.vector.scalar_tensor_tensor(
                        dwt_col, acc_col[:], cnt2_sb[:, t:t + 1],
                        corr_sb[:, t:t + 1],
                        op0=mybir.AluOpType.mult, op1=mybir.AluOpType.add,
                    )
                else:
                    nc.vector.tensor_scalar_mul(dwt_col, wt_col, DEN_SCALE)
                nc.vector.tensor_scalar_add(xt[:, E:E + 1], dwt_col, 0.0)
                # q column: q * wt = q * denwt / DEN_SCALE
                nc.vector.tensor_scalar(
                    xt[:, 0:E], qw_sb[:, t, :], dwt_col,
                    1.0 / DEN_SCALE, op0=mybir.AluOpType.mult,
                    op1=mybir.AluOpType.mult,
                )

                oh2 = oh_groups[og]
                # leftovers of this tile into previous tile's bank (closes it)
                if t >= 1:
                    nc.tensor.matmul(
                        bank_ps[t - 1][0:SLOTS, :], oh2[:, ogi, 0:SLOTS],
                        xt[:], start=False, stop=True,
                    )
                    nc.scalar.copy(
                        bank_sb[0:SLOTS, t - 1, :], bank_ps[t - 1][0:SLOTS, :]
                    )
                ps = pseg.tile([128, 129], F32)
                bank_ps[t] = ps
                last = t == NT - 1
                nc.tensor.matmul(
                    ps[0:SLOTS, :], oh2[:, ogi, SLOTS:2 * SLOTS], xt[:],
                    start=True, stop=last,
                )
                if last:
                    nc.scalar.copy(bank_sb[0:SLOTS, t, :], ps[0:SLOTS, :])

            # ---- phase C: gather num/den per row, dot with q ----
            NOG = (NT + GO - 1) // GO

            def ensure_ohg(og):
                if og >= NOG or ("g", og) in oh_groups:
                    return
                t0 = og * GO
                ogn = min(GO, NT - t0)
                ogt = ohgpool.tile([128, GO, 256], FP8)
                nc.sync.dma_start(
                    ogt[0:SLOTS, 0:ogn, :], ohs_gat[:, t0:t0 + ogn, :]
                )
                oh_groups[("g", og)] = ogt

            def phase_c(t):
                og, ogi = divmod(t, GO)
                if ogi == 0:
                    ensure_ohg(og)
                g2 = oh_groups[("g", og)]
                gp = pgat.tile([128, 129], F32)
                if t >= 1:
                    nc.tensor.matmul(
                        gp[:], g2[0:SLOTS, ogi, 0:128],
                        bank_sb[0:SLOTS, t - 1, :],
                        start=True, stop=False,
                    )
                    nc.tensor.matmul(
                        gp[:], g2[0:SLOTS, ogi, 128:256],
                        bank_sb[0:SLOTS, t, :],
                        start=False, stop=True,
                    )
                else:
                    nc.tensor.matmul(
                        gp[:], g2[0:SLOTS, ogi, 128:256],
                        bank_sb[0:SLOTS, t, :],
                        start=True, stop=True,
                    )
                nc.scalar.copy(den_sb[:, t:t + 1], gp[:, E:E + 1])
                # rnum = sum_e num[user] * q, fused multiply+accumulate
                pq = zpool.tile([128, E], BF16, tag="pq")
                nc.vector.scalar_tensor_tensor(
                    pq[:], gp[:, 0:E], 0.0, qw_sb[:, t, :],
                    op0=mybir.AluOpType.add, op1=mybir.AluOpType.mult,
                    accum_out=rn_sb[:, t:t + 1],
                )
                if ogi == GO - 1 or t == NT - 1:
                    # finalize this group: r = rnum / den, stream it out
                    t0 = og * GO
                    gn = t - t0 + 1
                    rec = small.tile([128, GO], F32, tag="rec")
                    nc.vector.reciprocal(rec[:, 0:gn], den_sb[:, t0:t0 + gn])
                    nc.vector.tensor_mul(
                        r_sb[:, t0:t0 + gn], rn_sb[:, t0:t0 + gn],
                        rec[:, 0:gn],
                    )
                    # Pool-engine (SWDGE) queue: its dependency wait must not
                    # head-of-line block the R stream DMAs on the SP queue.
                    # The final group rides SP (cheaper, queue is empty then).
                    eng = nc.sync if t == NT - 1 else nc.gpsimd
                    eng.dma_start(
                        r_out[:, t0:t0 + gn], r_sb[:, t0:t0 + gn]
                    )

            # interleave: phase C lags phase A by LAG tiles so its DMAs and
            # matmuls overlap the R stream instead of queueing after it
            LAG = 2
            for it in range(NT + LAG):
                if it < NT:
                    phase_a(it)
                if it >= LAG:
                    phase_c(it - LAG)

    nc.compile()
    return nc


# ----------------------------------------------------------------------------
# entry point
# ----------------------------------------------------------------------------

def kernel(users, items, R_ui, mask, w, item_emb, alpha, beta, gamma,
           _return_extras=False, _trace=False):
    users = np.asarray(users, np.int64)
    items = np.asarray(items, np.int64)
    R_ui = np.asarray(R_ui, np.float32)
    mask_b = np.asarray(mask)
    mask_f = mask_b.astype(np.float32)
    w = np.asarray(w, np.float32)
    item_emb = np.asarray(item_emb, np.float32)
    al = float(np.asarray(alpha).reshape(-1)[0])
    be = float(np.asarray(beta).reshape(-1)[0])
    ga = float(np.asarray(gamma).reshape(-1)[0])

    import time as _time

    t0 = _time.perf_counter()
    in_maps, metas, NT = _preprocess(users, items, R_ui, mask_f, w, item_emb)
    t1 = _time.perf_counter()
    nc = build_program(NT, al, be, ga)
    t2 = _time.perf_counter()
    res = run_bass_kernel_spmd(
        nc, in_maps, core_ids=list(range(N_CORES)), trace=_trace
    )
    t3 = _time.perf_counter()
    print(
        f"[kernel] preprocess {t1-t0:.1f}s  build+schedule {t2-t1:.1f}s  "
        f"compile+run {t3-t2:.1f}s"
    )

    n = users.shape[0]
    r = np.empty(n, np.float32)
    for c in range(N_CORES):
        p, nc_rows = metas[c]
        shard = res.results[c]["r_out"].T.reshape(-1)[:nc_rows]
        r[p] = shard
    if _return_extras:
        return r, res
    return r


# revision 52
# speedup vs baseline: 7.6902x; 1.0947x over previous
"""CDMF segment-reduce kernel for 8 Trainium2 NeuronCores.

Strategy
--------
Host (cheap, index-only + one big gather):
  * stable-sort rows by user id; cut the 100k rows into 8 shards at user
    boundaries ("expert-style sharding of user segments") so each core owns a
    disjoint user range -> no cross-core reduction needed at all.
  * pad every shard to NT*128 rows (mask=0 rows contribute exactly 0).
  * compact each row's valid (mask=1) seq-slices to the front and zero the
    dead tail; masked slices are never used by the math, so they are not
    transferred at all. Per-tile chunk counts CH[t] (shared across all
    cores, the program is SPMD) bound the compacted width.
  * pre-gather q = item_emb[items] per shard. R and q ship as fp8 e3m4
    with exact power-of-2 pre-scales (compensated in w / the den column),
    one-hots as fp8 0/1, the rest bf16.
  * transpose R into PE-friendly chunks RT[(c%2)*64+d, tile, j, row] so the
    feature contraction runs on the tensor engine.
  * build per-tile one-hot matrices mapping the 128 rows of a tile to
    user-slots of a PSUM "bank" (bank b = users first seen in tile b).
    Only SLOTS (max users/tile, rounded up to 32) slots are kept.

Device (one SPMD program on 8 cores, DMA-roofline bound):
  * PE: per tile, CH[t] accumulating matmuls lhsT=RT-chunk [K=128=(2 slices
    x 64 d), M=128 rows], rhs=w-pattern [128, W] -> Z [128 rows, W] in PSUM.
  * DVE: one tensor_scalar (max(Z,tau), sum-accum) -> acc; padded columns
    contribute exactly tau each, so den = DEN_SCALE*wt = acc*cnt2 + corr
    with host-precomputed per-row constants (no mask tensor at all).
  * PE one-hot matmuls accumulate per-user [sum wt*q | DEN_SCALE*sum wt]
    (N=129) into PSUM banks; ACT flushes each bank to SBUF (bf16).
  * transposed one-hot matmuls gather num[user]/den[user] back per row;
    fused scalar_tensor_tensor computes sum_e num*q; per-GO-group
    reciprocal+mul finalizes r and streams it out on the idle Pool queue.
  * phase C lags phase A by LAG tiles inside one program-order loop so its
    DMAs interleave with the R stream instead of queueing after it.
"""

import numpy as np
import ml_dtypes

import concourse.bass as bass
import concourse.tile as tile
from concourse import bacc, mybir
from concourse.bass_utils import run_bass_kernel_spmd

N_CORES = 8
TAU = 0.01
S = 50          # seq_len
D = 64          # n_features
E = 128         # emb_dim
NJ = S // 2     # max PE k-chunks per tile (2 s-slices of 64 features each)
GO = 16          # one-hot / mask / q tiles per DMA
F32 = mybir.dt.float32
BF16 = mybir.dt.bfloat16
FP8 = mybir.dt.float8e4
# R stream dtype: fp8 e3m4 with an exact 2x pre-scale (2 folded into R,
# 1/2 into w) -- 4 mantissa bits halve the Z error vs e4m3 at equal bytes.
R_DT = mybir.dt.float8e3
R_SCALE = 2.0
R_CLIP = 15.5 / R_SCALE
# q stream: same e3m4 trick with a 32x pre-scale (item_emb ~ N(0, 0.1^2)).
# The two q factors in r = (sum wt*q)(q)/den make r scale by 32^2; scaling
# the den column of X by 32^2 cancels it exactly (powers of two).
Q_DT = mybir.dt.float8e3
NP_Q = mybir.dt.np(Q_DT)
Q_SCALE = 32.0
Q_CLIP = 15.5 / Q_SCALE
DEN_SCALE = Q_SCALE * Q_SCALE

NP_BF16 = ml_dtypes.bfloat16
NP_FP8 = mybir.dt.np(FP8)
NP_R = mybir.dt.np(R_DT)


# R-tile DMA group plan: small groups at both ends to shorten the pipeline
# prologue (first compute waits on a small DMA) and epilogue (last tile's
# chain starts as early as possible).
def _group_plan(NT):
    front, tail = [1, 1, 2], [2, 2, 1, 1]
    if NT <= sum(front) + sum(tail):
        plan, acc = [], 0
        for g in front + tail:
            if acc >= NT:
                break
            plan.append(min(g, NT - acc))
            acc += plan[-1]
        return plan
    mid = NT - sum(front) - sum(tail)
    plan = front + [8] * (mid // 8)
    if mid % 8:
        plan.append(mid % 8)
    return plan + tail


# Data-dependent build parameters (set by _preprocess, read by
# build_program's defaults so `build_program(NT)` builds the same program
# that kernel() runs).
_BUILD_PARAMS = None


# ----------------------------------------------------------------------------
# host-side preprocessing
# ----------------------------------------------------------------------------

def _preprocess(users, items, R_ui, mask, w, item_emb):
    global _BUILD_PARAMS
    n = users.shape[0]
    perm = np.argsort(users, kind="stable")
    users_s = users[perm]

    # shard cuts at user boundaries
    cuts = [0]
    for c in range(1, N_CORES):
        t = round(c * n / N_CORES)
        while 0 < t < n and users_s[t] == users_s[t - 1]:
            t += 1
        cuts.append(min(t, n))
    cuts.append(n)
    sizes = [cuts[c + 1] - cuts[c] for c in range(N_CORES)]
    NT = max(1, int(np.ceil(max(sizes) / 128)))
    NPAD = NT * 128

    q_full = item_emb[items]  # [n, E]
    w_bf = np.asarray(w, NP_BF16)

    # ---- pass 1: per-core masks -> shared chunk counts + slot count ----
    mks = []
    cnts = []
    slots_needed = 1
    metas = []
    row_maps = []
    for c in range(N_CORES):
        lo, hi = cuts[c], cuts[c + 1]
        nc_rows = hi - lo
        p0 = perm[lo:hi]
        u0 = users_s[lo:hi]
        # reorder users within the shard by their max valid-count so tiles
        # hold rows of similar width -- shrinks the per-tile chunk padding.
        # Only run-contiguity per user matters for the bank structure.
        cr = mask[p0].sum(1).astype(np.int64)
        uniq0, inv0 = np.unique(u0, return_inverse=True)
        umax = np.zeros(len(uniq0), np.int64)
        np.maximum.at(umax, inv0, cr)
        order = np.lexsort((np.arange(len(u0)), u0, umax[inv0]))
        p = p0[order]

        mk = np.zeros((NPAD, S), np.float32)
        mk[:nc_rows] = mask[p]
        mks.append(mk)
        cnts.append(mk.sum(1).astype(np.int64))
        metas.append((p, nc_rows))

        u = np.empty(NPAD, np.int64)
        u[:nc_rows] = u0[order]
        u[nc_rows:] = u[nc_rows - 1] if nc_rows > 0 else 0
        uniq, first_idx, inv = np.unique(u, return_index=True,
                                         return_inverse=True)
        ft = first_idx // 128
        # rank of each user within its bank (ft is not monotonic in user
        # value after the reorder, so rank via a stable sort of ft)
        fo = np.argsort(ft, kind="stable")
        ft_s = ft[fo]
        slot_s = np.arange(len(uniq)) - np.searchsorted(ft_s, ft_s, "left")
        slot = np.empty(len(uniq), np.int64)
        slot[fo] = slot_s
        slots_needed = max(slots_needed, int(np.bincount(ft).max()))
        row_maps.append((ft[inv], slot[inv]))

    cnt_mat = np.stack(cnts)                        # [cores, NPAD]
    tile_max = cnt_mat.reshape(N_CORES, NT, 128).max(-1).max(0)  # [NT]
    CH = np.maximum(1, (tile_max + 1) // 2).astype(np.int64)     # chunks/tile
    OFF = np.concatenate([[0], np.cumsum(CH)])       # chunk offsets
    TOT = int(OFF[-1])
    Wt = 2 * CH                                      # Z columns per tile
    WOFF = np.concatenate([[0], np.cumsum(Wt)])
    WTOT = int(WOFF[-1])
    SLOTS = int(min(128, ((slots_needed + 31) // 32) * 32))
    assert slots_needed <= 128, "bank overflow"

    _BUILD_PARAMS = {
        "NT": NT,
        "CH": tuple(int(x) for x in CH),
        "SLOTS": SLOTS,
    }

    # ---- pass 2: per-core arrays ----
    in_maps = []
    for c in range(N_CORES):
        p, nc_rows = metas[c]
        mk = mks[c]
        cnt = cnts[c]

        # compact valid slices to the front of each row (stable order)
        Rp = np.zeros((NPAD, S, D), NP_R)
        Rp[:nc_rows] = np.clip(R_ui[p], -R_CLIP, R_CLIP) * R_SCALE
        vidx = np.argsort(mk <= 0, axis=1, kind="stable")  # valid first
        cmp = np.take_along_axis(Rp, vidx[:, :, None], axis=1)
        # zero the dead tail so padded Z columns are exactly 0 -> max()=tau
        cmp[np.arange(S)[None, :] >= cnt[:, None]] = 0

        RT = np.empty((128, TOT * 128), NP_R)
        for t in range(NT):
            ch = int(CH[t])
            blk = cmp[t * 128:(t + 1) * 128, 0:2 * ch, :]
            RT[:, OFF[t] * 128:OFF[t + 1] * 128] = (
                blk.reshape(128, ch, 2, D).transpose(2, 3, 1, 0)
                .reshape(128, ch * 128)
            )

        # the compacted mask (column c of tile t live iff c < cnt[row]) is
        # built on-device from cnt via an iota compare -- only cnt ships.
        cntw = np.ascontiguousarray(
            cnt.reshape(NT, 128).T, np.float32)      # [128, NT]
        iota50 = np.ascontiguousarray(
            np.broadcast_to(np.arange(S, dtype=np.float32), (128, S))
        ).astype(NP_BF16)
        # fast path: padded Z columns contribute exactly tau each, so
        # wt = cnt*(acc - (W-cnt)*tau); ship cnt and the correction,
        # both pre-scaled by DEN_SCALE for the X den column.
        cm = cntw.astype(np.float32)
        Wrow = (2 * CH).astype(np.float32)[None, :]
        cnt2 = np.ascontiguousarray(cm * DEN_SCALE)
        corr = np.ascontiguousarray(
            np.float32(TAU) * cm * (cm - Wrow) * DEN_SCALE, np.float32)

        qp = np.zeros((NPAD, E), NP_Q)
        qp[:nc_rows] = np.clip(q_full[p], -Q_CLIP, Q_CLIP) * Q_SCALE
        qw = np.ascontiguousarray(qp.reshape(NT, 128, E).transpose(1, 0, 2))

        # w-pattern for the PE feature contraction: [128=(c%2)*64+d, NJ, S]
        # (carries the 1/R_SCALE compensation -- exact, power of two)
        wpat = np.zeros((128, NJ, S), NP_BF16)
        wb = (w_bf.astype(np.float32) / R_SCALE).astype(NP_BF16)
        for sp in range(2):
            for j in range(NJ):
                wpat[sp * 64:(sp + 1) * 64, j, 2 * j + sp] = wb

        fr, sr = row_maps[c]
        ii = np.arange(NPAD)
        tt, kk = ii // 128, ii % 128
        own = fr == tt
        prev = fr == tt - 1
        assert np.all(own | prev), "user spans >2 tiles (unexpected padding)"

        seg = np.zeros((128, NT, 2 * SLOTS), NP_FP8)
        gat = np.zeros((SLOTS, NT, 256), NP_FP8)
        seg[kk[own], tt[own], SLOTS + sr[own]] = 1.0
        seg[kk[prev], tt[prev], sr[prev]] = 1.0
        gat[sr[own], tt[own], 128 + kk[own]] = 1.0
        gat[sr[prev], tt[prev], kk[prev]] = 1.0

        in_maps.append(
            {
                "RT": RT,
                "cntw": cntw,
                "cnt2": cnt2,
                "corr": corr,
                "iota50": iota50,
                "qw": qw,
                "wpat": wpat,
                "ohs_seg": seg,
                "ohs_gat": gat,
            }
        )
    return in_maps, metas, NT


# ----------------------------------------------------------------------------
# device program
# ----------------------------------------------------------------------------

def build_program(NT, alpha=1.0, beta=1.0, gamma=1.0, params=None):
    if params is None:
        params = _BUILD_PARAMS
    if params is None or params["NT"] != NT:
        params = {"NT": NT, "CH": (NJ,) * NT, "SLOTS": 128}
    CH = params["CH"]
    SLOTS = params["SLOTS"]
    OFF = [0]
    for ch in CH:
        OFF.append(OFF[-1] + ch)
    TOT = OFF[-1]
    WOFF = [2 * o for o in OFF]
    WTOT = 2 * TOT

    nc = bacc.Bacc(
        "TRN2", target_bir_lowering=False, debug=False, num_devices=N_CORES
    )

    RT = nc.dram_tensor("RT", [128, TOT * 128], R_DT, kind="ExternalInput")
    cntw = nc.dram_tensor("cntw", [128, NT], F32, kind="ExternalInput")
    cnt2 = nc.dram_tensor("cnt2", [128, NT], F32, kind="ExternalInput")
    corr = nc.dram_tensor("corr", [128, NT], F32, kind="ExternalInput")
    iota50 = nc.dram_tensor("iota50", [128, S], BF16, kind="ExternalInput")
    qw = nc.dram_tensor("qw", [128, NT, E], Q_DT, kind="ExternalInput")
    wpat = nc.dram_tensor("wpat", [128, NJ, S], BF16, kind="ExternalInput")
    ohs_seg = nc.dram_tensor("ohs_seg", [128, NT, 2 * SLOTS], FP8,
                             kind="ExternalInput")
    ohs_gat = nc.dram_tensor("ohs_gat", [SLOTS, NT, 256], FP8,
                             kind="ExternalInput")
    r_out = nc.dram_tensor("r_out", [128, NT], F32, kind="ExternalOutput")

    fast = (alpha == 1.0) and (beta == 1.0) and (gamma == 1.0)
    AF = mybir.ActivationFunctionType

    plan = _group_plan(NT)
    gstart = [0]
    for g in plan:
        gstart.append(gstart[-1] + g)
    MAXC = max(OFF[gstart[i + 1]] - OFF[gstart[i]] for i in range(len(plan)))

    with tile.TileContext(nc) as tc:
        with (
            tc.tile_pool(name="const", bufs=1) as constp,
            tc.tile_pool(name="rpool", bufs=3) as rpool,
            tc.tile_pool(name="zpool", bufs=4) as zpool,
            tc.tile_pool(name="mpool", bufs=2) as mpool,
            tc.tile_pool(name="small", bufs=8) as small,
            tc.tile_pool(name="xpool", bufs=6) as xpool,
            tc.tile_pool(name="ohpool", bufs=3) as ohpool,
            tc.tile_pool(name="ohgpool", bufs=4) as ohgpool,
            tc.tile_pool(name="banks", bufs=1) as bankp,
            tc.tile_pool(name="psum_z", bufs=2, space="PSUM") as pz,
            tc.tile_pool(name="psum_seg", bufs=3, space="PSUM") as pseg,
            tc.tile_pool(name="psum_gat", bufs=3, space="PSUM") as pgat,
        ):
            wpat_sb = constp.tile([128, NJ, S], BF16)
            nc.sync.dma_start(wpat_sb[:], wpat[:, :, :])
            iota_sb = constp.tile([128, S], BF16)
            nc.sync.dma_start(iota_sb[:], iota50[:, :])
            qw_sb = constp.tile([128, NT, E], Q_DT)
            cnt_sb = constp.tile([128, NT], F32)
            nc.sync.dma_start(cnt_sb[:], cntw[:, :])
            cnt2_sb = constp.tile([128, NT], F32)
            nc.sync.dma_start(cnt2_sb[:], cnt2[:, :])
            corr_sb = constp.tile([128, NT], F32)
            nc.sync.dma_start(corr_sb[:], corr[:, :])
            den_sb = constp.tile([128, NT], F32)
            wt_sb = constp.tile([128, NT], F32)
            rn_sb = constp.tile([128, NT], F32)
            r_sb = constp.tile([128, NT], F32)
            bank_sb = bankp.tile([128, NT, 129], BF16)

            r_groups = {}
            oh_groups = {}
            bank_ps = [None] * NT
            tile_group = []
            for gi_, g in enumerate(plan):
                tile_group += [gi_] * g

            def phase_a(t):
                g = tile_group[t]
                if t == gstart[g]:
                    t1 = gstart[g + 1]
                    c0, c1 = OFF[t], OFF[t1]
                    rg = rpool.tile([128, MAXC * 128], R_DT)
                    nc.sync.dma_start(
                        rg[:, 0:(c1 - c0) * 128], RT[:, c0 * 128:c1 * 128]
                    )
                    r_groups[g] = rg
                og, ogi = divmod(t, GO)
                if ogi == 0:
                    ogn = min(GO, NT - t)
                    osg = ohpool.tile([128, GO, 2 * SLOTS], FP8)
                    nc.sync.dma_start(
                        osg[:, 0:ogn, :], ohs_seg[:, t:t + ogn, :]
                    )
                    oh_groups[og] = osg
                    nc.sync.dma_start(
                        qw_sb[:, t:t + ogn, :], qw[:, t:t + ogn, :]
                    )

                W = 2 * CH[t]
                rg = r_groups[g]
                base = (OFF[t] - OFF[gstart[g]]) * 128
                zps = pz.tile([128, S], F32)
                for j in range(CH[t]):
                    nc.tensor.matmul(
                        zps[:, 0:W], rg[:, base + j * 128:base + (j + 1) * 128],
                        wpat_sb[:, j, 0:W],
                        start=(j == 0), stop=(j == CH[t] - 1),
                    )

                wt_col = wt_sb[:, t:t + 1]
                cnt_col = cnt_sb[:, t:t + 1]
                if fast:
                    # acc = sum_c max(z, tau); padded columns add exactly tau
                    # so den = DEN_SCALE*wt = acc*cnt2 + corr (see host)
                    acc_col = small.tile([128, 1], F32, tag="acc")
                    wp = zpool.tile([128, S], BF16)
                    nc.vector.tensor_scalar(
                        wp[:, 0:W], zps[:, 0:W], TAU, None,
                        op0=mybir.AluOpType.max, op1=mybir.AluOpType.add,
                        accum_out=acc_col[:],
                    )
                else:
                    mct = mpool.tile([128, S], BF16)
                    mcol = mct[:, 0:W]
                    nc.vector.tensor_scalar(
                        mcol, iota_sb[:, 0:W], cnt_col, 1.0,
                        op0=mybir.AluOpType.is_lt, op1=mybir.AluOpType.mult,
                    )
                    z = zpool.tile([128, S], F32, tag="zf32")
                    nc.vector.tensor_scalar_max(z[:, 0:W], zps[:, 0:W], TAU)
                    # z <- exp(alpha * ln z)   (z >= TAU > 0)
                    nc.scalar.activation(z[:, 0:W], z[:, 0:W], AF.Log)
                    nc.scalar.activation(z[:, 0:W], z[:, 0:W], AF.Exp,
                                         scale=float(alpha))
                    wp = zpool.tile([128, S], F32, tag="wpf32")
                    nc.vector.tensor_mul(wp[:, 0:W], z[:, 0:W], mcol)
                    a_col = small.tile([128, 1], F32)
                    nc.vector.tensor_reduce(
                        a_col[:], wp[:, 0:W], axis=mybir.AxisListType.X,
                        op=mybir.AluOpType.add,
                    )
                    # wt = (A^(1/alpha) * cnt^beta)^gamma
                    #    = exp(gamma*(ln(A)/alpha + beta*ln(cnt)))
                    la = small.tile([128, 1], F32)
                    nc.scalar.activation(la[:], a_col[:], AF.Log)
                    lc = small.tile([128, 1], F32)
                    nc.scalar.activation(lc[:], cnt_sb[:, t:t + 1], AF.Log)
                    nc.vector.scalar_tensor_tensor(
                        la[:], lc[:], float(alpha * beta), la[:],
                        op0=mybir.AluOpType.mult, op1=mybir.AluOpType.add,
                    )
                    nc.scalar.activation(
                        wt_col, la[:], AF.Exp, scale=float(gamma / alpha)
                    )

                # X_t = [wt*q | DEN_SCALE*wt]; dwt_col = DEN_SCALE*wt (f32)
                xt = xpool.tile([128, 129], BF16)
                dwt_col = wt_sb[:, t:t + 1]
                if fast:
                    nc.vector.scalar_tensor_tensor(
                        dwt_col, acc_col[:], cnt2_sb[:, t:t + 1],
                        corr_sb[:, t:t + 1],
                        op0=mybir.AluOpType.mult, op1=mybir.AluOpType.add,
                    )
                else:
                    nc.vector.tensor_scalar_mul(dwt_col, wt_col, DEN_SCALE)
                nc.vector.tensor_scalar_add(xt[:, E:E + 1], dwt_col, 0.0)
                # q column: q * wt = q * denwt / DEN_SCALE
                nc.vector.tensor_scalar(
                    xt[:, 0:E], qw_sb[:, t, :], dwt_col,
                    1.0 / DEN_SCALE, op0=mybir.AluOpType.mult,
                    op1=mybir.AluOpType.mult,
                )

                oh2 = oh_groups[og]
                # leftovers of this tile into previous tile's bank (closes it)
                if t >= 1:
                    nc.tensor.matmul(
                        bank_ps[t - 1][0:SLOTS, :], oh2[:, ogi, 0:SLOTS],
                        xt[:], start=False, stop=True,
                    )
                    nc.scalar.copy(
                        bank_sb[0:SLOTS, t - 1, :], bank_ps[t - 1][0:SLOTS, :]
                    )
                ps = pseg.tile([128, 129], F32)
                bank_ps[t] = ps
                last = t == NT - 1
                nc.tensor.matmul(
                    ps[0:SLOTS, :], oh2[:, ogi, SLOTS:2 * SLOTS], xt[:],
                    start=True, stop=last,
                )
                if last:
                    nc.scalar.copy(bank_sb[0:SLOTS, t, :], ps[0:SLOTS, :])

            # ---- phase C: gather num/den per row, dot with q ----
            NOG = (NT + GO - 1) // GO

            def ensure_ohg(og):
                if og >= NOG or ("g", og) in oh_groups:
                    return
                t0 = og * GO
                ogn = min(GO, NT - t0)
                ogt = ohgpool.tile([128, GO, 256], FP8)
                nc.sync.dma_start(
                    ogt[0:SLOTS, 0:ogn, :], ohs_gat[:, t0:t0 + ogn, :]
                )
                oh_groups[("g", og)] = ogt

            def phase_c(t):
                og, ogi = divmod(t, GO)
                if ogi == 0:
                    ensure_ohg(og)
                g2 = oh_groups[("g", og)]
                gp = pgat.tile([128, 129], F32)
                if t >= 1:
                    nc.tensor.matmul(
                        gp[:], g2[0:SLOTS, ogi, 0:128],
                        bank_sb[0:SLOTS, t - 1, :],
                        start=True, stop=False,
                    )
                    nc.tensor.matmul(
                        gp[:], g2[0:SLOTS, ogi, 128:256],
                        bank_sb[0:SLOTS, t, :],
                        start=False, stop=True,
                    )
                else:
                    nc.tensor.matmul(
                        gp[:], g2[0:SLOTS, ogi, 128:256],
                        bank_sb[0:SLOTS, t, :],
                        start=True, stop=True,
                    )
                nc.scalar.copy(den_sb[:, t:t + 1], gp[:, E:E + 1])
                # rnum = sum_e num[user] * q, fused multiply+accumulate
                pq = zpool.tile([128, E], BF16, tag="pq")
                nc.vector.scalar_tensor_tensor(
                    pq[:], gp[:, 0:E], 0.0, qw_sb[:, t, :],
                    op0=mybir.AluOpType.add, op1=mybir.AluOpType.mult,
                    accum_out=rn_sb[:, t:t + 1],
                )
                if ogi == GO - 1 or t == NT - 1:
                    # finalize this group: r = rnum / den, stream it out
                    t0 = og * GO
                    gn = t - t0 + 1
                    rec = small.tile([128, GO], F32, tag="rec")
                    nc.vector.reciprocal(rec[:, 0:gn], den_sb[:, t0:t0 + gn])
                    nc.vector.tensor_mul(
                        r_sb[:, t0:t0 + gn], rn_sb[:, t0:t0 + gn],
                        rec[:, 0:gn],
                    )
                    # Pool-engine (SWDGE) queue: its dependency wait must not
                    # head-of-line block the R stream DMAs on the SP queue.
                    # The final group rides SP (cheaper, queue is empty then).
                    eng = nc.sync if t == NT - 1 else nc.gpsimd
                    eng.dma_start(
                        r_out[:, t0:t0 + gn], r_sb[:, t0:t0 + gn]
                    )

            # interleave: phase C lags phase A by LAG tiles so its DMAs and
            # matmuls overlap the R stream instead of queueing after it
            LAG = 2
            for it in range(NT + LAG):
                if it < NT:
                    phase_a(it)
                if it >= LAG:
                    phase_c(it - LAG)

    nc.compile()
    return nc


# ----------------------------------------------------------------------------
# entry point
# ----------------------------------------------------------------------------

def kernel(users, items, R_ui, mask, w, item_emb, alpha, beta, gamma,
           _return_extras=False, _trace=False):
    users = np.asarray(users, np.int64)
    items = np.asarray(items, np.int64)
    R_ui = np.asarray(R_ui, np.float32)
    mask_b = np.asarray(mask)
    mask_f = mask_b.astype(np.float32)
    w = np.asarray(w, np.float32)
    item_emb = np.asarray(item_emb, np.float32)
    al = float(np.asarray(alpha).reshape(-1)[0])
    be = float(np.asarray(beta).reshape(-1)[0])
    ga = float(np.asarray(gamma).reshape(-1)[0])

    import time as _time

    t0 = _time.perf_counter()
    in_maps, metas, NT = _preprocess(users, items, R_ui, mask_f, w, item_emb)
    t1 = _time.perf_counter()
    nc = build_program(NT, al, be, ga)
    t2 = _time.perf_counter()
    res = run_bass_kernel_spmd(
        nc, in_maps, core_ids=list(range(N_CORES)), trace=_trace
    )
    t3 = _time.perf_counter()
    print(
        f"[kernel] preprocess {t1-t0:.1f}s  build+schedule {t2-t1:.1f}s  "
        f"compile+run {t3-t2:.1f}s"
    )

    n = users.shape[0]
    r = np.empty(n, np.float32)
    for c in range(N_CORES):
        p, nc_rows = metas[c]
        shard = res.results[c]["r_out"].T.reshape(-1)[:nc_rows]
        r[p] = shard
    if _return_extras:
        return r, res
    return r
